# revision 15
# baseline (speedup 1.0000x reference)
"""Trainium2 Bass kernel for nn_InvLocalPatOrientConvolution.

Computation:
  1. Host: synthesize the 160-channel 5x5x5 conv filter (scaled x32), split
     weights and input into fp8-e4m3 hi/lo pairs (w = w_hi + w_lo exactly at
     fp8 resolution; x = x_hi + x_lo/16), lay out per-core operands.
  2. Device (8 NeuronCores, SPMD): VALID 3D conv as fp8 DoubleRow PE matmuls.
     The conv contraction (125 taps x 16 cin = 2000 rows) is packed onto the
     full 128 SBUF partitions: the host materializes (j,k)-shifted copies of
     x so that rows = 8 (j,k)-taps x 16 cin per tile (3 tiles = taps 0..23),
     plus an 80-row "runt" tensor holding tap 24 = (j=4,k=4) with the i-shift
     baked in (rows = 5 i-planes x 16 cin).  Per chunk and per output group:
       - 16 main DRs: (w_hi, w_lo) x broadcast x_hi   (full-precision w)
       - ~9 xlo DRs:  w/16 x x_lo pairs (tile pairs / plane pairs / runt)
     Output-channel groups: A = conv channels (e*10+l) < 128 (128 cols);
     B = the remaining 32 channels pre-contracted with the so3 grid on the
     host (cols = 108 grid-space outputs, accumulated straight into the
     mt=3 so3 psum tile, so no gb matmul / cb copy on device).
     SO(3) grid pooling (relu-weighted second-moment ratio) stays fp16 on
     the PE; the x32 weight scale rides through the a-psum and is folded
     into the relu scale vector.
     Sharding: batch (2) x output-X-slabs (4) -> 8 cores.
     Chunk xr=0 uses a compact 80-row x_lo layout (15 matmuls/group instead
     of 9) so the first chunk does not wait for the big shifted-copy DMAs.
  3. Host: gather per-core slabs into the full (2,16,36,36,36) output.
"""

import os
import sys

for _p in ("/root/.axon_site/_ro/trn_rl_repo", "/opt/trn_rl_repo"):
    if os.path.isdir(_p) and _p not in sys.path:
        sys.path.insert(0, _p)

import numpy as np
import ml_dtypes

import concourse.mybir as mybir
from concourse import bacc
from concourse.tile import TileContext
from concourse.bass_utils import run_bass_kernel_spmd

# Problem constants (hardcoded per harness contract)
ORDER = 2
KS = 5            # conv kernel size
CIN = 16
COUT = 16
EPS = 1e-16
S = 10            # wigner rows
B = 2
D_IN = 40         # input spatial
D_OUT = 36        # output spatial
SLAB = 9          # output X planes per core (36/4)
SLAB_IN = SLAB + KS - 1   # 13 input X planes per core
NCORES = 8
YB = 12           # y-block per chunk
NCHUNK = YB * D_OUT       # 432 columns per matmul chunk
WSCALE = 32.0     # filter pre-scale so fp8 hi/lo split keeps precision

# weight slot layout in wt/wtb tensors: [128, NWSLOT, 2, M]
#   slots 0..14   main (i*3+t): (w_hi, w_lo) for taps t*8..t*8+7 at plane i
#   slots 15..19  xlo tile-pair (i): (w16[i,t0], w16[i,t1])
#   slots 20..21  xlo t2 plane-pair (i=0,2): (w16[i,t2], w16[i+1,t2])
#   slot  22      xlo t2 single (i=4): (w16[4,t2], 0)
NW_MAIN = 0
NW_XPAIR = 15
NW_T2PAIR = 20
NW_T2SINGLE = 22
NWSLOT = 23
NCOLB = 112       # conv-B grid-space cols (108 + 4 zero pad: ldweights
                  # row width must be a multiple of 16)
# runt tensors wr/wrb: [80, 17, 2, M]
#   slot 0 main runt (w_hi, w_lo); slot 1 xlo runt (w16, 0)
#   slots 2..16 the 80-row xlo scheme for chunk xr=0: q = k*3 + i0/2
NR_XLO80 = 2
NRSLOT = 17

F8 = mybir.dt.float8e4
F16 = mybir.dt.float16
F32 = mybir.dt.float32
NPF8 = ml_dtypes.float8_e4m3
DR = mybir.MatmulPerfMode.DoubleRow

_prog_cache = {}


def _conv_chunk(nc, ps, wtx, wrx, xm, xlm, xrh, xrl, xlo80, xr, y0,
                stop_last=True):
    """Emit the conv matmuls of one chunk into psum ps (col count = wtx M)."""
    first = True

    def mm(lhsT, rhs, stop=False):
        nonlocal first
        nc.tensor.matmul(ps[:], lhsT, rhs, start=first, stop=stop,
                         perf_mode=DR)
        first = False

    # main: (w_hi, w_lo) x broadcast x_hi; tile-outer so chunk 0 can start
    # as soon as the first tile's planes have landed
    for t in range(3):
        for i in range(KS):
            rhs = xm[:, t, xr + i:xr + i + 1, y0:y0 + YB, :] \
                .broadcast_to([128, 2, YB, D_OUT])
            mm(wtx[:, NW_MAIN + i * 3 + t, :, :], rhs)
    mm(wrx[:, 0, :, :],
       xrh[:, xr:xr + 1, y0:y0 + YB, :].broadcast_to([80, 2, YB, D_OUT]))

    if xr == 0:
        # compact 80-row xlo: rows (j,cin), k by column offset, i plane-pairs
        q = 0
        for k in range(KS):
            for i0 in (0, 2, 4):
                rhs = xlo80[:, i0:i0 + 2, y0:y0 + YB, k:k + D_OUT]
                mm(wrx[:, NR_XLO80 + q, :, :], rhs,
                   stop=(stop_last and q == 14))
                q += 1
        return

    # xlo: w/16 x x_lo, pairing (t0,t1) tiles per plane
    for i in range(KS):
        mm(wtx[:, NW_XPAIR + i, :, :], xlm[:, 0:2, xr + i, y0:y0 + YB, :])
    # xlo t2 tile: plane pairs (0,1), (2,3); plane 4 solo
    for i in (0, 2):
        s = xr + i
        mm(wtx[:, NW_T2PAIR + i // 2, :, :], xlm[:, 2, s:s + 2, y0:y0 + YB, :])
    s = xr + 4
    mm(wtx[:, NW_T2SINGLE, :, :],
       xlm[:, 2, s:s + 1, y0:y0 + YB, :].broadcast_to([128, 2, YB, D_OUT]))
    # xlo runt tap
    mm(wrx[:, 1, :, :],
       xrl[:, xr:xr + 1, y0:y0 + YB, :].broadcast_to([80, 2, YB, D_OUT]),
       stop=stop_last)


def _build_program():
    """Build the SPMD device program (identical on all 8 cores)."""
    nc = bacc.Bacc("TRN2")

    xm_d = nc.dram_tensor("xm", [128, 3, SLAB_IN, D_OUT, D_OUT], F8,
                          kind="ExternalInput")
    xlm_d = nc.dram_tensor("xlm", [128, 3, SLAB_IN, D_OUT, D_OUT], F8,
                           kind="ExternalInput")
    xrh_d = nc.dram_tensor("xrh", [80, SLAB, D_OUT, D_OUT], F8,
                           kind="ExternalInput")
    xrl_d = nc.dram_tensor("xrl", [80, SLAB, D_OUT, D_OUT], F8,
                           kind="ExternalInput")
    xlo80_d = nc.dram_tensor("xlo80", [80, 6, D_OUT, D_IN], F8,
                             kind="ExternalInput")
    wt_d = nc.dram_tensor("wt", [128, NWSLOT, 2, 128], F8,
                          kind="ExternalInput")
    wtb_d = nc.dram_tensor("wtb", [128, NWSLOT, 2, NCOLB], F8,
                           kind="ExternalInput")
    wr_d = nc.dram_tensor("wr", [80, NRSLOT, 2, 128], F8,
                          kind="ExternalInput")
    wrb_d = nc.dram_tensor("wrb", [80, NRSLOT, 2, NCOLB], F8,
                           kind="ExternalInput")
    ga_d = nc.dram_tensor("ga", [128, 4, 108], F16, kind="ExternalInput")
    wnd_d = nc.dram_tensor("wnd", [108, 64], F16, kind="ExternalInput")
    one8_d = nc.dram_tensor("one8", [108, 4, 2, 16], F8, kind="ExternalInput")
    wvec_d = nc.dram_tensor("wvec", [108, 1], F32, kind="ExternalInput")
    bias_d = nc.dram_tensor("bias", [16, 1], F32, kind="ExternalInput")
    y_d = nc.dram_tensor("y", [16, SLAB, D_OUT, D_OUT], F32,
                         kind="ExternalOutput")

    chunks = [(xr, cy) for xr in range(SLAB) for cy in range(3)]

    with TileContext(nc) as tc:
        with tc.tile_pool(name="const", bufs=1) as cpool, \
             tc.tile_pool(name="work", bufs=4) as wpool, \
             tc.tile_pool(name="casb", bufs=4) as capool, \
             tc.tile_pool(name="rrel", bufs=10) as rpool, \
             tc.tile_pool(name="conv_ps", bufs=2, space="PSUM") as conv_pool, \
             tc.tile_pool(name="a_ps", bufs=3, space="PSUM") as a_pool, \
             tc.tile_pool(name="nd_ps", bufs=2, space="PSUM") as nd_pool, \
             tc.tile_pool(name="den_ps", bufs=1, space="PSUM") as den_pool:

            # ---- resident constants + x tap-copies (all planes SBUF-resident)
            xm = cpool.tile([128, 3, SLAB_IN, D_OUT, D_OUT], F8, tag="xm")
            xlm = cpool.tile([128, 3, SLAB_IN, D_OUT, D_OUT], F8, tag="xlm")
            xrh = cpool.tile([80, SLAB, D_OUT, D_OUT], F8, tag="xrh")
            xrl = cpool.tile([80, SLAB, D_OUT, D_OUT], F8, tag="xrl")
            xlo80 = cpool.tile([80, 6, D_OUT, D_IN], F8, tag="xlo80")
            wt = cpool.tile([128, NWSLOT, 2, 128], F8, tag="wt")
            wtb = cpool.tile([128, NWSLOT, 2, NCOLB], F8, tag="wtb")
            wr = cpool.tile([80, NRSLOT, 2, 128], F8, tag="wr")
            wrb = cpool.tile([80, NRSLOT, 2, NCOLB], F8, tag="wrb")
            gat = cpool.tile([128, 4, 108], F16)
            wndt = cpool.tile([108, 64], F16)
            one8t = cpool.tile([108, 4, 2, 16], F8)
            wvect = cpool.tile([108, 1], F32)
            biast = cpool.tile([16, 1], F32)
            dma_engs = [nc.sync, nc.scalar, nc.gpsimd]

            def _ld_plane(xt, xt_d, t, p0, p1, q):
                dma_engs[q % 3].dma_start(
                    out=xt[:, t, p0:p1].rearrange("p a b c -> p (a b c)"),
                    in_=xt_d[:, t, p0:p1].rearrange("p a b c -> p (a b c)"))

            def _ld_runt(xt, xt_d, p0, p1, q):
                dma_engs[q % 3].dma_start(
                    out=xt[:, p0:p1].rearrange("p a b c -> p (a b c)"),
                    in_=xt_d[:, p0:p1].rearrange("p a b c -> p (a b c)"))

            def _flat(ap):
                return ap.rearrange("p a b c -> p (a b c)")

            # PE warm-up: dummy fp32 matmuls keep the PE busy (and its
            # p-state ramping) while the chunk-0 DMAs land.
            warm = cpool.tile([128, NCHUNK], F32, tag="warm")
            nc.vector.memset(warm[:], 0.0)
            for wi in range(8):
                wps = conv_pool.tile([128, NCHUNK], F32, tag="cps")
                nc.tensor.matmul(wps[:], warm[:, 0:128], warm[:],
                                 start=True, stop=True)

            # chunk-0 critical data first, in big DMAs (per-DMA overhead is
            # ~1.3us).  Chunk-0 critical: weights, xrh plane 0, xm tiles
            # planes 0-4, xlo80.  Then consts, xlm planes 0-4, then the rest
            # in need order.
            nc.sync.dma_start(out=_flat(wt[:, 0:NWSLOT]),
                              in_=_flat(wt_d[:, 0:NWSLOT]))
            nc.scalar.dma_start(out=_flat(wtb[:, 0:NWSLOT]),
                                in_=_flat(wtb_d[:, 0:NWSLOT]))
            nc.gpsimd.dma_start(out=_flat(wr[:]), in_=_flat(wr_d[:]))
            nc.gpsimd.dma_start(out=_flat(wrb[:]), in_=_flat(wrb_d[:]))
            _ld_runt(xrh, xrh_d, 0, 1, 2)
            _ld_plane(xm, xm_d, 0, 0, KS, 0)
            nc.gpsimd.dma_start(
                out=xlo80.rearrange("p a b c -> p (a b c)"),
                in_=xlo80_d.rearrange("p a b c -> p (a b c)"))
            _ld_plane(xm, xm_d, 1, 0, KS, 1)
            _ld_plane(xm, xm_d, 2, 0, KS, 0)
            nc.sync.dma_start(out=gat[:], in_=ga_d[:])
            nc.scalar.dma_start(out=wndt[:], in_=wnd_d[:])
            nc.scalar.dma_start(out=one8t[:], in_=one8_d[:])
            nc.sync.dma_start(out=wvect[:], in_=wvec_d[:])
            nc.sync.dma_start(out=biast[:], in_=bias_d[:])
            for t in range(3):
                _ld_plane(xlm, xlm_d, t, 0, KS, t)
            # tail, in need order (chunk xr needs x plane xr+4, runt plane xr)
            _ld_runt(xrh, xrh_d, 1, 3, 0)
            _ld_runt(xrl, xrl_d, 1, 3, 1)
            for t in range(3):
                _ld_plane(xm, xm_d, t, KS, KS + 2, t)
            for t in range(3):
                _ld_plane(xlm, xlm_d, t, KS, KS + 2, t)
            _ld_runt(xrh, xrh_d, 3, 5, 0)
            _ld_runt(xrl, xrl_d, 3, 5, 1)
            for t in range(3):
                _ld_plane(xm, xm_d, t, KS + 2, KS + 4, t)
            for t in range(3):
                _ld_plane(xlm, xlm_d, t, KS + 2, KS + 4, t)
            _ld_runt(xrh, xrh_d, 5, SLAB, 0)
            _ld_runt(xrl, xrl_d, 5, SLAB, 1)
            for t in range(3):
                _ld_plane(xm, xm_d, t, KS + 4, SLAB_IN, t)
            for t in range(3):
                _ld_plane(xlm, xlm_d, t, KS + 4, SLAB_IN, t)

            pending = None
            for (xr, cy) in chunks:
                y0 = cy * YB
                # ---- conv A (128 conv channels), compensated fp8 DR
                cps = conv_pool.tile([128, NCHUNK], F32, tag="cps")
                _conv_chunk(nc, cps, wt, wr, xm, xlm, xrh, xrl, xlo80,
                            xr, y0)
                ca = capool.tile([128, NCHUNK], F16, tag="ca")
                nc.scalar.copy(ca[:], cps[:])

                # ---- conv B: grid-space output accumulated into the mt=3
                # so3 psum tile (so3's gb matmul is folded into the weights)
                aps3f = a_pool.tile([NCOLB, NCHUNK], F32, tag="aps")
                _conv_chunk(nc, aps3f, wtb, wrb, xm, xlm, xrh, xrl, xlo80,
                            xr, y0, stop_last=False)
                aps3 = aps3f[0:108]

                # ---- so3 grid + relu/square (moments lag one chunk)
                rrels, r2s = [], []
                for mt in range(4):
                    if mt < 3:
                        aps = a_pool.tile([108, NCHUNK], F32, tag="aps")
                        nc.tensor.matmul(aps[:], gat[:, mt, :], ca[:],
                                         start=True, stop=True)
                    else:
                        aps = aps3
                        nc.tensor.matmul(aps, gat[:, mt, :], ca[:],
                                         start=False, stop=True)
                    wrel = rpool.tile([108, NCHUNK], F16, tag="rrel")
                    apv = aps[:] if mt < 3 else aps
                    nc.scalar.activation(wrel[:], apv,
                                         mybir.ActivationFunctionType.Relu,
                                         scale=wvect[:, 0:1])
                    w8 = rpool.tile([108, 2, NCHUNK], F8, tag="w8")
                    nc.gpsimd.tensor_copy(w8[:, 0, :], wrel[:])
                    nc.vector.tensor_sub(w8[:, 1, :], wrel[:], w8[:, 0, :])
                    r2 = rpool.tile([108, NCHUNK], F16, tag="r2")
                    nc.vector.tensor_mul(r2[:], wrel[:], wrel[:])
                    rrels.append(w8)
                    r2s.append(r2)
                nd_ps = nd_pool.tile([16, NCHUNK], F32, tag="nd")
                den_ps = den_pool.tile([16, NCHUNK], F32, tag="dn")
                if pending is not None:
                    _emit_moments(nc, wndt, one8t, biast, wpool, y_d, pending)
                pending = (nd_ps, den_ps, rrels, r2s, xr, y0)
            if pending is not None:
                _emit_moments(nc, wndt, one8t, biast, wpool, y_d, pending)

    nc.finalize()
    return nc


def _emit_moments(nc, wndt, one8t, biast, wpool, y_d, st):
    """Emit the 8 moment matmuls + finalize + store for a chunk whose grid
    stage (a/relu/square) was already emitted."""
    nd_ps, den_ps, rrels, r2s, xr, y0 = st
    for mt in range(4):
        wnd_g = wndt[:, mt * 16:(mt + 1) * 16]
        nc.tensor.matmul(nd_ps[:], wnd_g, r2s[mt][:],
                         start=(mt == 0), stop=(mt == 3))
        nc.tensor.matmul(den_ps[:], one8t[:, mt, :, :], rrels[mt][:],
                         start=(mt == 0), stop=(mt == 3), perf_mode=DR)

    num_sb = wpool.tile([16, NCHUNK], F32, tag="num_sb")
    nc.scalar.copy(num_sb[:], nd_ps[:])
    den_sb = wpool.tile([16, NCHUNK], F32, tag="den_sb")
    nc.scalar.activation(den_sb[:], den_ps[:],
         mybir.ActivationFunctionType.Copy,
         bias=EPS)
    recip = wpool.tile([16, NCHUNK], F32, tag="recip")
    nc.vector.reciprocal(recip[:], den_sb[:])
    out_sb = wpool.tile([16, NCHUNK], F32, tag="out_sb")
    nc.vector.tensor_mul(out_sb[:], num_sb[:], recip[:])
    nc.vector.tensor_scalar_add(out_sb[:], out_sb[:], biast[:, 0:1])
    dst = y_d[:, xr].rearrange("p a b -> p (a b)")[
        :, y0 * D_OUT:(y0 + YB) * D_OUT]
    nc.sync.dma_start(out=dst, in_=out_sb[:])


def _synthesize_filter(weight, zeroweight, basis_functions, wig_w, wig_b):
    """Replicate the reference's kernel synthesis in fp32 numpy.

    Returns kern6[l, e, d, i, j, k] of shape (10, 16, 16, 5, 5, 5)."""
    zero_ext = np.concatenate(
        [zeroweight[None, None],
         np.zeros((ORDER ** 2 - 1, 1, CIN, COUT), weight.dtype)], axis=0)
    wfull = np.concatenate([zero_ext, weight], axis=1)       # (4, 10, 16, 16)
    wg = wfull[wig_w]                                        # (10, 10, 16, 16)
    bg = basis_functions[wig_b]                              # (10, 10, 5, 5, 5)
    kern6 = np.einsum("lred,lrijk->ledijk", wg, bg)          # (10,16,16,5,5,5)
    return np.ascontiguousarray(kern6.astype(np.float32))


def _pack_weights(w6, ncols):
    """Pack a [125, CIN, ncols] fp32 filter into wt/wr-style slot tensors."""
    w_hi = w6.astype(NPF8)
    w_lo = (w6 - w_hi.astype(np.float32)).astype(NPF8)
    w_x16 = (w6 / 16).astype(NPF8)

    wt_arr = np.zeros((128, NWSLOT, 2, ncols), NPF8)
    for i in range(KS):
        for t in range(3):
            for jkl in range(8):
                jk = t * 8 + jkl
                j, k = jk // KS, jk % KS
                tap = i * 25 + j * 5 + k
                r0 = jkl * CIN
                wt_arr[r0:r0 + CIN, NW_MAIN + i * 3 + t, 0, :] = w_hi[tap]
                wt_arr[r0:r0 + CIN, NW_MAIN + i * 3 + t, 1, :] = w_lo[tap]
        for sl, t in ((0, 0), (1, 1)):
            for jkl in range(8):
                jk = t * 8 + jkl
                j, k = jk // KS, jk % KS
                tap = i * 25 + j * 5 + k
                r0 = jkl * CIN
                wt_arr[r0:r0 + CIN, NW_XPAIR + i, sl, :] = w_x16[tap]

    def _t2_block(dst_slot, sl, i):
        for jkl in range(8):
            jk = 16 + jkl
            j, k = jk // KS, jk % KS
            tap = i * 25 + j * 5 + k
            r0 = jkl * CIN
            wt_arr[r0:r0 + CIN, dst_slot, sl, :] = w_x16[tap]
    for pi, i in enumerate((0, 2)):
        _t2_block(NW_T2PAIR + pi, 0, i)
        _t2_block(NW_T2PAIR + pi, 1, i + 1)
    _t2_block(NW_T2SINGLE, 0, 4)

    wr_arr = np.zeros((80, NRSLOT, 2, ncols), NPF8)
    for i in range(KS):
        tap = i * 25 + 4 * 5 + 4
        r0 = i * CIN
        wr_arr[r0:r0 + CIN, 0, 0, :] = w_hi[tap]
        wr_arr[r0:r0 + CIN, 0, 1, :] = w_lo[tap]
        wr_arr[r0:r0 + CIN, 1, 0, :] = w_x16[tap]
    # 80-row xlo slots for chunk xr=0: rows (j*16+cin), q = k*3 + i0/2
    q = 0
    for k in range(KS):
        for i0 in (0, 2, 4):
            for j in range(KS):
                r0 = j * CIN
                wr_arr[r0:r0 + CIN, NR_XLO80 + q, 0, :] = \
                    w_x16[i0 * 25 + j * 5 + k]
                if i0 + 1 < KS:
                    wr_arr[r0:r0 + CIN, NR_XLO80 + q, 1, :] = \
                        w_x16[(i0 + 1) * 25 + j * 5 + k]
            q += 1
    return wt_arr, wr_arr


def _host_prep(x, weight, zeroweight, bias, so3basisgrid, w_i,
               basis_functions, wig_w, wig_b):
    kern6 = _synthesize_filter(weight, zeroweight, basis_functions, wig_w, wig_b)

    # w6[tap, cin, col] with tap = i*25 + j*5 + k, col = e*10+l; scaled x32
    w6 = np.ascontiguousarray(
        kern6.transpose(3, 4, 5, 2, 1, 0).reshape(125, CIN, 160)
    ).astype(np.float32) * WSCALE

    g2 = so3basisgrid.reshape(27, S).astype(np.float32)      # raw grid
    g2t = g2.T                                               # [l, mln]

    # B channels (cols 128..159) pre-contracted with the grid: 108 outputs
    # B row r: r=0,1 -> (e12, l8+r); r=2+10*m+l -> (e13+m, l)
    gbmap = np.zeros((32, 108), np.float32)
    for r in range(32):
        if r < 2:
            e, l = 12, 8 + r
        else:
            e, l = 13 + (r - 2) // S, (r - 2) % S
        el2 = e - 12
        gbmap[r, el2 * 27:(el2 + 1) * 27] = g2t[l]
    w6b = np.einsum("tcb,bn->tcn", w6[:, :, 128:], gbmap)
    w6b = np.concatenate(
        [w6b, np.zeros((125, CIN, NCOLB - 108), np.float32)], axis=2)

    wt_arr, wr_arr = _pack_weights(w6[:, :, :128], 128)
    wtb_arr, wrb_arr = _pack_weights(np.ascontiguousarray(w6b), NCOLB)

    # A-tile so3 lhsT: ga[p, mt, el2*27+mln]; p = e*10+l (p < 128)
    ga = np.zeros((128, 4, 108), np.float16)
    for mt in range(4):
        for el2 in range(4):
            e = 4 * mt + el2
            for l in range(S):
                p = e * S + l
                if p < 128:
                    ga[p, mt, el2 * 27:(el2 + 1) * 27] = g2t[l]

    # weighted-moment lhsT: wnd[(el2*27+mln), mt*16+e], e = 4mt+el2
    w_flat = np.asarray(w_i, np.float32)[(np.arange(27) // 3) % 3]
    wnd = np.zeros((108, 4, 16), np.float16)
    one8 = np.zeros((108, 4, 2, 16), NPF8)
    for mt in range(4):
        for el2 in range(4):
            e = 4 * mt + el2
            wnd[el2 * 27:(el2 + 1) * 27, mt, e] = \
                (1.0 / w_flat).astype(np.float16)
            one8[el2 * 27:(el2 + 1) * 27, mt, :, e] = 1.0
    wnd = wnd.reshape(108, 64)
    # the x32 filter scale rides through the a-psum; fold 1/32 into the
    # relu scale so wrel comes out in natural units
    wvec = (np.tile(w_flat, 4) / WSCALE).reshape(108, 1).astype(np.float32)

    bias_arr = np.asarray(bias, np.float32).reshape(16, 1)

    x = np.asarray(x, np.float32)
    xh_all = x.astype(NPF8)
    xl_all = ((x - xh_all.astype(np.float32)) * 16).astype(NPF8)

    in_maps = []
    for c in range(NCORES):
        b, qq = divmod(c, 4)
        p0 = qq * SLAB
        # windowed views: win[cin, p, y, z, j, k] = x[cin, p0+p, y+j, z+k]
        def _wins(arr):
            sl = arr[b, :, p0:p0 + SLAB_IN]        # (16, 13, 40, 40)
            s0, s1, s2, s3 = sl.strides
            return np.lib.stride_tricks.as_strided(
                sl, (CIN, SLAB_IN, D_OUT, D_OUT, KS, KS),
                (s0, s1, s2, s3, s2, s3))
        xm_arr = np.empty((128, 3, SLAB_IN, D_OUT, D_OUT), NPF8)
        xrun = np.empty((2, 80, SLAB, D_OUT, D_OUT), NPF8)
        for hl, arr in enumerate((xh_all, xl_all)):
            w = _wins(arr)
            # main tiles: row (jkl*16+cin) of tile t = tap jk = t*8+jkl
            wv = w.transpose(4, 5, 0, 1, 2, 3).reshape(
                25, CIN, SLAB_IN, D_OUT, D_OUT)
            tiles = wv[:24].reshape(3, 8, CIN, SLAB_IN, D_OUT, D_OUT) \
                .reshape(3, 128, SLAB_IN, D_OUT, D_OUT) \
                .transpose(1, 0, 2, 3, 4)
            if hl == 0:
                xm_arr[:] = tiles
            else:
                xlm_arr = np.ascontiguousarray(tiles)
            # runt: row (i*16+cin) at out-plane xr = x[cin, xr+i, y+4, z+4]
            rw = w[:, :, :, :, 4, 4]               # (16, 13, 36, 36)
            for i in range(KS):
                xrun[hl, i * CIN:(i + 1) * CIN] = rw[:, i:i + SLAB]
        # 80-row xlo for chunk 0: rows (j*16+cin), planes 0..5, z full
        xlo80 = np.empty((80, 6, D_OUT, D_IN), NPF8)
        sl = xl_all[b, :, p0:p0 + 6]               # (16, 6, 40, 40)
        for j in range(KS):
            xlo80[j * CIN:(j + 1) * CIN] = sl[:, :, j:j + D_OUT, :]
        in_maps.append({
            "xm": np.ascontiguousarray(xm_arr),
            "xlm": xlm_arr,
            "xrh": np.ascontiguousarray(xrun[0]),
            "xrl": np.ascontiguousarray(xrun[1]),
            "xlo80": np.ascontiguousarray(xlo80),
            "wt": wt_arr,
            "wtb": wtb_arr,
            "wr": wr_arr,
            "wrb": wrb_arr,
            "ga": np.ascontiguousarray(ga),
            "wnd": np.ascontiguousarray(wnd),
            "one8": np.ascontiguousarray(one8),
            "wvec": np.ascontiguousarray(wvec),
            "bias": bias_arr,
        })
    return in_maps


def _run(inputs, trace=False, **run_kwargs):
    inputs = {k: np.asarray(v) for k, v in inputs.items()}
    in_maps = _host_prep(**inputs)
    if "nc" not in _prog_cache:
        _prog_cache["nc"] = _build_program()
    nc = _prog_cache["nc"]
    try:
        res = run_bass_kernel_spmd(nc, in_maps, core_ids=list(range(NCORES)),
                                   trace=trace, **run_kwargs)
    except ModuleNotFoundError as e:
        if "axon_hooks" not in str(e):
            raise
        # Tracing requested (e.g. BASS_TRACE=1) but this axon client has no
        # NTFF profile hook - rerun with tracing disabled.
        os.environ["BASS_NEVER_TRACE"] = "1"
        res = run_bass_kernel_spmd(nc, in_maps, core_ids=list(range(NCORES)),
                                   trace=False, **run_kwargs)
    out = np.empty((B, COUT, D_OUT, D_OUT, D_OUT), np.float32)
    for c in range(NCORES):
        b, qq = divmod(c, 4)
        out[b, :, qq * SLAB:(qq + 1) * SLAB] = res.results[c]["y"]
    return out, res


def kernel(**inputs):
    out, _ = _run(inputs)
    return out


# revision 16
# speedup vs baseline: 1.0146x; 1.0146x over previous
"""Trainium2 Bass kernel for nn_InvLocalPatOrientConvolution.

Computation:
  1. Host: synthesize the 160-channel 5x5x5 conv filter (scaled x32), split
     weights and input into fp8-e4m3 hi/lo pairs (w = w_hi + w_lo exactly at
     fp8 resolution; x = x_hi + x_lo/16), lay out per-core operands.
  2. Device (8 NeuronCores, SPMD): VALID 3D conv as fp8 DoubleRow PE matmuls.
     The conv contraction (125 taps x 16 cin = 2000 rows) is packed onto the
     full 128 SBUF partitions: the host materializes (j,k)-shifted copies of
     x so that rows = 8 (j,k)-taps x 16 cin per tile (3 tiles = taps 0..23),
     plus an 80-row "runt" tensor holding tap 24 = (j=4,k=4) with the i-shift
     baked in (rows = 5 i-planes x 16 cin).  Per chunk and per output group:
       - 16 main DRs: (w_hi, w_lo) x broadcast x_hi   (full-precision w)
       - ~9 xlo DRs:  w/16 x x_lo pairs (tile pairs / plane pairs / runt)
     Output-channel groups: A = conv channels (e*10+l) < 128 (128 cols);
     B = the remaining 32 channels pre-contracted with the so3 grid on the
     host (cols = 108 grid-space outputs, accumulated straight into the
     mt=3 so3 psum tile, so no gb matmul / cb copy on device).
     SO(3) grid pooling (relu-weighted second-moment ratio) stays fp16 on
     the PE; the x32 weight scale rides through the a-psum and is folded
     into the relu scale vector.
     Sharding: batch (2) x output-X-slabs (4) -> 8 cores.
     Chunk xr=0 uses a compact 80-row x_lo layout (15 matmuls/group instead
     of 9) so the first chunk does not wait for the big shifted-copy DMAs.
  3. Host: gather per-core slabs into the full (2,16,36,36,36) output.
"""

import os
import sys

for _p in ("/root/.axon_site/_ro/trn_rl_repo", "/opt/trn_rl_repo"):
    if os.path.isdir(_p) and _p not in sys.path:
        sys.path.insert(0, _p)

import numpy as np
import ml_dtypes

import concourse.mybir as mybir
from concourse import bacc
from concourse.tile import TileContext
from concourse.bass_utils import run_bass_kernel_spmd

# Problem constants (hardcoded per harness contract)
ORDER = 2
KS = 5            # conv kernel size
CIN = 16
COUT = 16
EPS = 1e-16
S = 10            # wigner rows
B = 2
D_IN = 40         # input spatial
D_OUT = 36        # output spatial
SLAB = 9          # output X planes per core (36/4)
SLAB_IN = SLAB + KS - 1   # 13 input X planes per core
NCORES = 8
YB = 12           # y-block per chunk
NCHUNK = YB * D_OUT       # 432 columns per matmul chunk
WSCALE = 32.0     # filter pre-scale so fp8 hi/lo split keeps precision

# weight slot layout in wt/wtb tensors: [128, NWSLOT, 2, M]
#   slots 0..14   main (i*3+t): (w_hi, w_lo) for taps t*8..t*8+7 at plane i
#   slots 15..19  xlo tile-pair (i): (w16[i,t0], w16[i,t1])
#   slots 20..21  xlo t2 plane-pair (i=0,2): (w16[i,t2], w16[i+1,t2])
#   slot  22      xlo t2 single (i=4): (w16[4,t2], 0)
NW_MAIN = 0
NW_XPAIR = 15
NW_T2PAIR = 20
NW_T2SINGLE = 22
NWSLOT = 23
NCOLB = 112       # conv-B grid-space cols (108 + 4 zero pad: ldweights
                  # row width must be a multiple of 16)
# runt tensors wr/wrb: [80, 17, 2, M]
#   slot 0 main runt (w_hi, w_lo); slot 1 xlo runt (w16, 0)
#   slots 2..16 the 80-row xlo scheme for chunk xr=0: q = k*3 + i0/2
NR_XLO80 = 2
NRSLOT = 17

F8 = mybir.dt.float8e4
F16 = mybir.dt.float16
F32 = mybir.dt.float32
NPF8 = ml_dtypes.float8_e4m3
DR = mybir.MatmulPerfMode.DoubleRow

_prog_cache = {}


def _conv_chunk(nc, ps, wtx, wrx, xm, xlm, xrh, xrl, xlo80, xr, y0,
                stop_last=True):
    """Emit the conv matmuls of one chunk into psum ps (col count = wtx M)."""
    first = True

    def mm(lhsT, rhs, stop=False):
        nonlocal first
        nc.tensor.matmul(ps[:], lhsT, rhs, start=first, stop=stop,
                         perf_mode=DR)
        first = False

    # main: (w_hi, w_lo) x broadcast x_hi; tile-outer so chunk 0 can start
    # as soon as the first tile's planes have landed
    for t in range(3):
        for i in range(KS):
            rhs = xm[:, t, xr + i:xr + i + 1, y0:y0 + YB, :] \
                .broadcast_to([128, 2, YB, D_OUT])
            mm(wtx[:, NW_MAIN + i * 3 + t, :, :], rhs)
    mm(wrx[:, 0, :, :],
       xrh[:, xr:xr + 1, y0:y0 + YB, :].broadcast_to([80, 2, YB, D_OUT]))

    if xr == 0:
        # compact 80-row xlo: rows (j,cin), k by column offset, i plane-pairs
        q = 0
        for k in range(KS):
            for i0 in (0, 2, 4):
                rhs = xlo80[:, i0:i0 + 2, y0:y0 + YB, k:k + D_OUT]
                mm(wrx[:, NR_XLO80 + q, :, :], rhs,
                   stop=(stop_last and q == 14))
                q += 1
        return

    # xlo: w/16 x x_lo, pairing (t0,t1) tiles per plane
    for i in range(KS):
        mm(wtx[:, NW_XPAIR + i, :, :], xlm[:, 0:2, xr + i, y0:y0 + YB, :])
    # xlo t2 tile: plane pairs (0,1), (2,3); plane 4 solo
    for i in (0, 2):
        s = xr + i
        mm(wtx[:, NW_T2PAIR + i // 2, :, :], xlm[:, 2, s:s + 2, y0:y0 + YB, :])
    s = xr + 4
    mm(wtx[:, NW_T2SINGLE, :, :],
       xlm[:, 2, s:s + 1, y0:y0 + YB, :].broadcast_to([128, 2, YB, D_OUT]))
    # xlo runt tap
    mm(wrx[:, 1, :, :],
       xrl[:, xr:xr + 1, y0:y0 + YB, :].broadcast_to([80, 2, YB, D_OUT]),
       stop=stop_last)


def _build_program():
    """Build the SPMD device program (identical on all 8 cores)."""
    nc = bacc.Bacc("TRN2")

    xm_d = nc.dram_tensor("xm", [128, 3, SLAB_IN, D_OUT, D_OUT], F8,
                          kind="ExternalInput")
    xlm_d = nc.dram_tensor("xlm", [128, 3, SLAB_IN, D_OUT, D_OUT], F8,
                           kind="ExternalInput")
    xrh_d = nc.dram_tensor("xrh", [80, SLAB, D_OUT, D_OUT], F8,
                           kind="ExternalInput")
    xrl_d = nc.dram_tensor("xrl", [80, SLAB, D_OUT, D_OUT], F8,
                           kind="ExternalInput")
    xlo80_d = nc.dram_tensor("xlo80", [80, 6, D_OUT, D_IN], F8,
                             kind="ExternalInput")
    wt_d = nc.dram_tensor("wt", [128, NWSLOT, 2, 128], F8,
                          kind="ExternalInput")
    wtb_d = nc.dram_tensor("wtb", [128, NWSLOT, 2, NCOLB], F8,
                           kind="ExternalInput")
    wr_d = nc.dram_tensor("wr", [80, NRSLOT, 2, 128], F8,
                          kind="ExternalInput")
    wrb_d = nc.dram_tensor("wrb", [80, NRSLOT, 2, NCOLB], F8,
                           kind="ExternalInput")
    ga_d = nc.dram_tensor("ga", [128, 4, 108], F16, kind="ExternalInput")
    wnd_d = nc.dram_tensor("wnd", [108, 64], F16, kind="ExternalInput")
    one8_d = nc.dram_tensor("one8", [108, 4, 2, 16], F8, kind="ExternalInput")
    wvec_d = nc.dram_tensor("wvec", [108, 1], F32, kind="ExternalInput")
    bias_d = nc.dram_tensor("bias", [16, 1], F32, kind="ExternalInput")
    y_d = nc.dram_tensor("y", [16, SLAB, D_OUT, D_OUT], F32,
                         kind="ExternalOutput")

    chunks = [(xr, cy) for xr in range(SLAB) for cy in range(3)]

    with TileContext(nc) as tc:
        with tc.tile_pool(name="const", bufs=1) as cpool, \
             tc.tile_pool(name="work", bufs=4) as wpool, \
             tc.tile_pool(name="casb", bufs=4) as capool, \
             tc.tile_pool(name="rrel", bufs=10) as rpool, \
             tc.tile_pool(name="conv_ps", bufs=2, space="PSUM") as conv_pool, \
             tc.tile_pool(name="a_ps", bufs=3, space="PSUM") as a_pool, \
             tc.tile_pool(name="nd_ps", bufs=2, space="PSUM") as nd_pool, \
             tc.tile_pool(name="den_ps", bufs=1, space="PSUM") as den_pool:

            # ---- resident constants + x tap-copies (all planes SBUF-resident)
            xm = cpool.tile([128, 3, SLAB_IN, D_OUT, D_OUT], F8, tag="xm")
            xlm = cpool.tile([128, 3, SLAB_IN, D_OUT, D_OUT], F8, tag="xlm")
            xrh = cpool.tile([80, SLAB, D_OUT, D_OUT], F8, tag="xrh")
            xrl = cpool.tile([80, SLAB, D_OUT, D_OUT], F8, tag="xrl")
            xlo80 = cpool.tile([80, 6, D_OUT, D_IN], F8, tag="xlo80")
            wt = cpool.tile([128, NWSLOT, 2, 128], F8, tag="wt")
            wtb = cpool.tile([128, NWSLOT, 2, NCOLB], F8, tag="wtb")
            wr = cpool.tile([80, NRSLOT, 2, 128], F8, tag="wr")
            wrb = cpool.tile([80, NRSLOT, 2, NCOLB], F8, tag="wrb")
            gat = cpool.tile([128, 4, 108], F16)
            wndt = cpool.tile([108, 64], F16)
            one8t = cpool.tile([108, 4, 2, 16], F8)
            wvect = cpool.tile([108, 1], F32)
            biast = cpool.tile([16, 1], F32)
            dma_engs = [nc.sync, nc.scalar, nc.gpsimd]

            def _ld_plane(xt, xt_d, t, p0, p1, q):
                dma_engs[q % 3].dma_start(
                    out=xt[:, t, p0:p1].rearrange("p a b c -> p (a b c)"),
                    in_=xt_d[:, t, p0:p1].rearrange("p a b c -> p (a b c)"))

            def _ld_runt(xt, xt_d, p0, p1, q):
                dma_engs[q % 3].dma_start(
                    out=xt[:, p0:p1].rearrange("p a b c -> p (a b c)"),
                    in_=xt_d[:, p0:p1].rearrange("p a b c -> p (a b c)"))

            def _flat(ap):
                return ap.rearrange("p a b c -> p (a b c)")

            # PE warm-up: dummy fp32 matmuls keep the PE busy (and its
            # p-state ramping) while the chunk-0 DMAs land.
            warm = cpool.tile([128, NCHUNK], F32, tag="warm")
            nc.vector.memset(warm[:], 0.0)
            for wi in range(8):
                wps = conv_pool.tile([128, NCHUNK], F32, tag="cps")
                nc.tensor.matmul(wps[:], warm[:, 0:128], warm[:],
                                 start=True, stop=True)

            # chunk-0 critical data first, in big DMAs (per-DMA overhead is
            # ~1.3us).  Chunk-0 critical: weights, xrh plane 0, xm tiles
            # planes 0-4, xlo80.  Then consts, xlm planes 0-4, then the rest
            # in need order.
            # gpsimd gets only the three small weight DMAs, then stays free
            # for the per-chunk w8hi casts; everything else rides sync/scalar
            nc.gpsimd.dma_start(out=_flat(wr[:]), in_=_flat(wr_d[:]))
            nc.gpsimd.dma_start(out=_flat(wrb[:]), in_=_flat(wrb_d[:]))
            nc.gpsimd.dma_start(
                out=xlo80.rearrange("p a b c -> p (a b c)"),
                in_=xlo80_d.rearrange("p a b c -> p (a b c)"))
            nc.sync.dma_start(out=_flat(wt[:, 0:NWSLOT]),
                              in_=_flat(wt_d[:, 0:NWSLOT]))
            nc.scalar.dma_start(out=_flat(wtb[:, 0:NWSLOT]),
                                in_=_flat(wtb_d[:, 0:NWSLOT]))
            _ld_plane(xm, xm_d, 0, 0, KS, 0)
            _ld_plane(xm, xm_d, 1, 0, KS, 1)
            _ld_plane(xm, xm_d, 2, 0, KS, 0)
            nc.scalar.dma_start(out=gat[:], in_=ga_d[:])
            nc.scalar.dma_start(out=wvect[:], in_=wvec_d[:])
            _ld_runt(xrh, xrh_d, 0, 1, 1)
            nc.scalar.dma_start(out=wndt[:], in_=wnd_d[:])
            nc.scalar.dma_start(out=one8t[:], in_=one8_d[:])
            nc.scalar.dma_start(out=biast[:], in_=bias_d[:])
            _ld_plane(xlm, xlm_d, 0, 0, KS, 0)
            _ld_plane(xlm, xlm_d, 1, 0, KS, 1)
            _ld_plane(xlm, xlm_d, 2, 0, KS, 0)
            # tail, in need order (chunk xr needs x plane xr+4, runt plane xr)
            _ld_runt(xrh, xrh_d, 1, 3, 0)
            _ld_runt(xrl, xrl_d, 1, 3, 1)
            for t in range(3):
                _ld_plane(xm, xm_d, t, KS, KS + 2, t % 2)
            for t in range(3):
                _ld_plane(xlm, xlm_d, t, KS, KS + 2, (t + 1) % 2)
            _ld_runt(xrh, xrh_d, 3, 5, 0)
            _ld_runt(xrl, xrl_d, 3, 5, 1)
            for t in range(3):
                _ld_plane(xm, xm_d, t, KS + 2, KS + 4, t % 2)
            for t in range(3):
                _ld_plane(xlm, xlm_d, t, KS + 2, KS + 4, (t + 1) % 2)
            _ld_runt(xrh, xrh_d, 5, SLAB, 0)
            _ld_runt(xrl, xrl_d, 5, SLAB, 1)
            for t in range(3):
                _ld_plane(xm, xm_d, t, KS + 4, SLAB_IN, t % 2)
            for t in range(3):
                _ld_plane(xlm, xlm_d, t, KS + 4, SLAB_IN, (t + 1) % 2)

            pending = None
            for (xr, cy) in chunks:
                y0 = cy * YB
                # ---- conv A (128 conv channels), compensated fp8 DR
                cps = conv_pool.tile([128, NCHUNK], F32, tag="cps")
                _conv_chunk(nc, cps, wt, wr, xm, xlm, xrh, xrl, xlo80,
                            xr, y0)
                ca = capool.tile([128, NCHUNK], F16, tag="ca")
                nc.scalar.copy(ca[:], cps[:])

                # ---- conv B: grid-space output accumulated into the mt=3
                # so3 psum tile (so3's gb matmul is folded into the weights)
                aps3f = a_pool.tile([NCOLB, NCHUNK], F32, tag="aps")
                _conv_chunk(nc, aps3f, wtb, wrb, xm, xlm, xrh, xrl, xlo80,
                            xr, y0, stop_last=False)
                aps3 = aps3f[0:108]

                # ---- so3 grid + relu/square (moments lag one chunk)
                rrels, r2s = [], []
                for mt in range(4):
                    if mt < 3:
                        aps = a_pool.tile([108, NCHUNK], F32, tag="aps")
                        nc.tensor.matmul(aps[:], gat[:, mt, :], ca[:],
                                         start=True, stop=True)
                    else:
                        aps = aps3
                        nc.tensor.matmul(aps, gat[:, mt, :], ca[:],
                                         start=False, stop=True)
                    wrel = rpool.tile([108, NCHUNK], F16, tag="rrel")
                    apv = aps[:] if mt < 3 else aps
                    nc.scalar.activation(wrel[:], apv,
                                         mybir.ActivationFunctionType.Relu,
                                         scale=wvect[:, 0:1])
                    w8 = rpool.tile([108, 2, NCHUNK], F8, tag="w8")
                    nc.gpsimd.tensor_copy(w8[:, 0, :], wrel[:])
                    nc.vector.tensor_sub(w8[:, 1, :], wrel[:], w8[:, 0, :])
                    r2 = rpool.tile([108, NCHUNK], F16, tag="r2")
                    nc.vector.tensor_mul(r2[:], wrel[:], wrel[:])
                    rrels.append(w8)
                    r2s.append(r2)
                nd_ps = nd_pool.tile([16, NCHUNK], F32, tag="nd")
                den_ps = den_pool.tile([16, NCHUNK], F32, tag="dn")
                if pending is not None:
                    _emit_moments(nc, wndt, one8t, biast, wpool, y_d, pending)
                pending = (nd_ps, den_ps, rrels, r2s, xr, y0)
            if pending is not None:
                _emit_moments(nc, wndt, one8t, biast, wpool, y_d, pending)

    nc.finalize()
    return nc


def _emit_moments(nc, wndt, one8t, biast, wpool, y_d, st):
    """Emit the 8 moment matmuls + finalize + store for a chunk whose grid
    stage (a/relu/square) was already emitted."""
    nd_ps, den_ps, rrels, r2s, xr, y0 = st
    for mt in range(4):
        wnd_g = wndt[:, mt * 16:(mt + 1) * 16]
        nc.tensor.matmul(nd_ps[:], wnd_g, r2s[mt][:],
                         start=(mt == 0), stop=(mt == 3))
        nc.tensor.matmul(den_ps[:], one8t[:, mt, :, :], rrels[mt][:],
                         start=(mt == 0), stop=(mt == 3), perf_mode=DR)

    num_sb = wpool.tile([16, NCHUNK], F32, tag="num_sb")
    nc.scalar.copy(num_sb[:], nd_ps[:])
    den_sb = wpool.tile([16, NCHUNK], F32, tag="den_sb")
    nc.scalar.activation(den_sb[:], den_ps[:],
         mybir.ActivationFunctionType.Copy,
         bias=EPS)
    recip = wpool.tile([16, NCHUNK], F32, tag="recip")
    nc.vector.reciprocal(recip[:], den_sb[:])
    out_sb = wpool.tile([16, NCHUNK], F32, tag="out_sb")
    nc.vector.tensor_mul(out_sb[:], num_sb[:], recip[:])
    nc.vector.tensor_scalar_add(out_sb[:], out_sb[:], biast[:, 0:1])
    dst = y_d[:, xr].rearrange("p a b -> p (a b)")[
        :, y0 * D_OUT:(y0 + YB) * D_OUT]
    nc.sync.dma_start(out=dst, in_=out_sb[:])


def _synthesize_filter(weight, zeroweight, basis_functions, wig_w, wig_b):
    """Replicate the reference's kernel synthesis in fp32 numpy.

    Returns kern6[l, e, d, i, j, k] of shape (10, 16, 16, 5, 5, 5)."""
    zero_ext = np.concatenate(
        [zeroweight[None, None],
         np.zeros((ORDER ** 2 - 1, 1, CIN, COUT), weight.dtype)], axis=0)
    wfull = np.concatenate([zero_ext, weight], axis=1)       # (4, 10, 16, 16)
    wg = wfull[wig_w]                                        # (10, 10, 16, 16)
    bg = basis_functions[wig_b]                              # (10, 10, 5, 5, 5)
    kern6 = np.einsum("lred,lrijk->ledijk", wg, bg)          # (10,16,16,5,5,5)
    return np.ascontiguousarray(kern6.astype(np.float32))


def _pack_weights(w6, ncols):
    """Pack a [125, CIN, ncols] fp32 filter into wt/wr-style slot tensors."""
    w_hi = w6.astype(NPF8)
    w_lo = (w6 - w_hi.astype(np.float32)).astype(NPF8)
    w_x16 = (w6 / 16).astype(NPF8)

    wt_arr = np.zeros((128, NWSLOT, 2, ncols), NPF8)
    for i in range(KS):
        for t in range(3):
            for jkl in range(8):
                jk = t * 8 + jkl
                j, k = jk // KS, jk % KS
                tap = i * 25 + j * 5 + k
                r0 = jkl * CIN
                wt_arr[r0:r0 + CIN, NW_MAIN + i * 3 + t, 0, :] = w_hi[tap]
                wt_arr[r0:r0 + CIN, NW_MAIN + i * 3 + t, 1, :] = w_lo[tap]
        for sl, t in ((0, 0), (1, 1)):
            for jkl in range(8):
                jk = t * 8 + jkl
                j, k = jk // KS, jk % KS
                tap = i * 25 + j * 5 + k
                r0 = jkl * CIN
                wt_arr[r0:r0 + CIN, NW_XPAIR + i, sl, :] = w_x16[tap]

    def _t2_block(dst_slot, sl, i):
        for jkl in range(8):
            jk = 16 + jkl
            j, k = jk // KS, jk % KS
            tap = i * 25 + j * 5 + k
            r0 = jkl * CIN
            wt_arr[r0:r0 + CIN, dst_slot, sl, :] = w_x16[tap]
    for pi, i in enumerate((0, 2)):
        _t2_block(NW_T2PAIR + pi, 0, i)
        _t2_block(NW_T2PAIR + pi, 1, i + 1)
    _t2_block(NW_T2SINGLE, 0, 4)

    wr_arr = np.zeros((80, NRSLOT, 2, ncols), NPF8)
    for i in range(KS):
        tap = i * 25 + 4 * 5 + 4
        r0 = i * CIN
        wr_arr[r0:r0 + CIN, 0, 0, :] = w_hi[tap]
        wr_arr[r0:r0 + CIN, 0, 1, :] = w_lo[tap]
        wr_arr[r0:r0 + CIN, 1, 0, :] = w_x16[tap]
    # 80-row xlo slots for chunk xr=0: rows (j*16+cin), q = k*3 + i0/2
    q = 0
    for k in range(KS):
        for i0 in (0, 2, 4):
            for j in range(KS):
                r0 = j * CIN
                wr_arr[r0:r0 + CIN, NR_XLO80 + q, 0, :] = \
                    w_x16[i0 * 25 + j * 5 + k]
                if i0 + 1 < KS:
                    wr_arr[r0:r0 + CIN, NR_XLO80 + q, 1, :] = \
                        w_x16[(i0 + 1) * 25 + j * 5 + k]
            q += 1
    return wt_arr, wr_arr


def _host_prep(x, weight, zeroweight, bias, so3basisgrid, w_i,
               basis_functions, wig_w, wig_b):
    kern6 = _synthesize_filter(weight, zeroweight, basis_functions, wig_w, wig_b)

    # w6[tap, cin, col] with tap = i*25 + j*5 + k, col = e*10+l; scaled x32
    w6 = np.ascontiguousarray(
        kern6.transpose(3, 4, 5, 2, 1, 0).reshape(125, CIN, 160)
    ).astype(np.float32) * WSCALE

    g2 = so3basisgrid.reshape(27, S).astype(np.float32)      # raw grid
    g2t = g2.T                                               # [l, mln]

    # B channels (cols 128..159) pre-contracted with the grid: 108 outputs
    # B row r: r=0,1 -> (e12, l8+r); r=2+10*m+l -> (e13+m, l)
    gbmap = np.zeros((32, 108), np.float32)
    for r in range(32):
        if r < 2:
            e, l = 12, 8 + r
        else:
            e, l = 13 + (r - 2) // S, (r - 2) % S
        el2 = e - 12
        gbmap[r, el2 * 27:(el2 + 1) * 27] = g2t[l]
    w6b = np.einsum("tcb,bn->tcn", w6[:, :, 128:], gbmap)
    w6b = np.concatenate(
        [w6b, np.zeros((125, CIN, NCOLB - 108), np.float32)], axis=2)

    wt_arr, wr_arr = _pack_weights(w6[:, :, :128], 128)
    wtb_arr, wrb_arr = _pack_weights(np.ascontiguousarray(w6b), NCOLB)

    # A-tile so3 lhsT: ga[p, mt, el2*27+mln]; p = e*10+l (p < 128)
    ga = np.zeros((128, 4, 108), np.float16)
    for mt in range(4):
        for el2 in range(4):
            e = 4 * mt + el2
            for l in range(S):
                p = e * S + l
                if p < 128:
                    ga[p, mt, el2 * 27:(el2 + 1) * 27] = g2t[l]

    # weighted-moment lhsT: wnd[(el2*27+mln), mt*16+e], e = 4mt+el2
    w_flat = np.asarray(w_i, np.float32)[(np.arange(27) // 3) % 3]
    wnd = np.zeros((108, 4, 16), np.float16)
    one8 = np.zeros((108, 4, 2, 16), NPF8)
    for mt in range(4):
        for el2 in range(4):
            e = 4 * mt + el2
            wnd[el2 * 27:(el2 + 1) * 27, mt, e] = \
                (1.0 / w_flat).astype(np.float16)
            one8[el2 * 27:(el2 + 1) * 27, mt, :, e] = 1.0
    wnd = wnd.reshape(108, 64)
    # the x32 filter scale rides through the a-psum; fold 1/32 into the
    # relu scale so wrel comes out in natural units
    wvec = (np.tile(w_flat, 4) / WSCALE).reshape(108, 1).astype(np.float32)

    bias_arr = np.asarray(bias, np.float32).reshape(16, 1)

    x = np.asarray(x, np.float32)
    xh_all = x.astype(NPF8)
    xl_all = ((x - xh_all.astype(np.float32)) * 16).astype(NPF8)

    in_maps = []
    for c in range(NCORES):
        b, qq = divmod(c, 4)
        p0 = qq * SLAB
        # windowed views: win[cin, p, y, z, j, k] = x[cin, p0+p, y+j, z+k]
        def _wins(arr):
            sl = arr[b, :, p0:p0 + SLAB_IN]        # (16, 13, 40, 40)
            s0, s1, s2, s3 = sl.strides
            return np.lib.stride_tricks.as_strided(
                sl, (CIN, SLAB_IN, D_OUT, D_OUT, KS, KS),
                (s0, s1, s2, s3, s2, s3))
        xm_arr = np.empty((128, 3, SLAB_IN, D_OUT, D_OUT), NPF8)
        xrun = np.empty((2, 80, SLAB, D_OUT, D_OUT), NPF8)
        for hl, arr in enumerate((xh_all, xl_all)):
            w = _wins(arr)
            # main tiles: row (jkl*16+cin) of tile t = tap jk = t*8+jkl
            wv = w.transpose(4, 5, 0, 1, 2, 3).reshape(
                25, CIN, SLAB_IN, D_OUT, D_OUT)
            tiles = wv[:24].reshape(3, 8, CIN, SLAB_IN, D_OUT, D_OUT) \
                .reshape(3, 128, SLAB_IN, D_OUT, D_OUT) \
                .transpose(1, 0, 2, 3, 4)
            if hl == 0:
                xm_arr[:] = tiles
            else:
                xlm_arr = np.ascontiguousarray(tiles)
            # runt: row (i*16+cin) at out-plane xr = x[cin, xr+i, y+4, z+4]
            rw = w[:, :, :, :, 4, 4]               # (16, 13, 36, 36)
            for i in range(KS):
                xrun[hl, i * CIN:(i + 1) * CIN] = rw[:, i:i + SLAB]
        # 80-row xlo for chunk 0: rows (j*16+cin), planes 0..5, z full
        xlo80 = np.empty((80, 6, D_OUT, D_IN), NPF8)
        sl = xl_all[b, :, p0:p0 + 6]               # (16, 6, 40, 40)
        for j in range(KS):
            xlo80[j * CIN:(j + 1) * CIN] = sl[:, :, j:j + D_OUT, :]
        in_maps.append({
            "xm": np.ascontiguousarray(xm_arr),
            "xlm": xlm_arr,
            "xrh": np.ascontiguousarray(xrun[0]),
            "xrl": np.ascontiguousarray(xrun[1]),
            "xlo80": np.ascontiguousarray(xlo80),
            "wt": wt_arr,
            "wtb": wtb_arr,
            "wr": wr_arr,
            "wrb": wrb_arr,
            "ga": np.ascontiguousarray(ga),
            "wnd": np.ascontiguousarray(wnd),
            "one8": np.ascontiguousarray(one8),
            "wvec": np.ascontiguousarray(wvec),
            "bias": bias_arr,
        })
    return in_maps


def _run(inputs, trace=False, **run_kwargs):
    inputs = {k: np.asarray(v) for k, v in inputs.items()}
    in_maps = _host_prep(**inputs)
    if "nc" not in _prog_cache:
        _prog_cache["nc"] = _build_program()
    nc = _prog_cache["nc"]
    try:
        res = run_bass_kernel_spmd(nc, in_maps, core_ids=list(range(NCORES)),
                                   trace=trace, **run_kwargs)
    except ModuleNotFoundError as e:
        if "axon_hooks" not in str(e):
            raise
        # Tracing requested (e.g. BASS_TRACE=1) but this axon client has no
        # NTFF profile hook - rerun with tracing disabled.
        os.environ["BASS_NEVER_TRACE"] = "1"
        res = run_bass_kernel_spmd(nc, in_maps, core_ids=list(range(NCORES)),
                                   trace=False, **run_kwargs)
    out = np.empty((B, COUT, D_OUT, D_OUT, D_OUT), np.float32)
    for c in range(NCORES):
        b, qq = divmod(c, 4)
        out[b, :, qq * SLAB:(qq + 1) * SLAB] = res.results[c]["y"]
    return out, res


def kernel(**inputs):
    out, _ = _run(inputs)
    return out


# revision 17
# speedup vs baseline: 1.0336x; 1.0187x over previous
"""Trainium2 Bass kernel for nn_InvLocalPatOrientConvolution.

Computation:
  1. Host: synthesize the 160-channel 5x5x5 conv filter (scaled x32), split
     weights and input into fp8-e4m3 hi/lo pairs (w = w_hi + w_lo exactly at
     fp8 resolution; x = x_hi + x_lo/16), lay out per-core operands.
  2. Device (8 NeuronCores, SPMD): VALID 3D conv as fp8 DoubleRow PE matmuls.
     The conv contraction (125 taps x 16 cin = 2000 rows) is packed onto the
     full 128 SBUF partitions: the host materializes (j,k)-shifted copies of
     x so that rows = 8 (j,k)-taps x 16 cin per tile (3 tiles = taps 0..23),
     plus an 80-row "runt" tensor holding tap 24 = (j=4,k=4) with the i-shift
     baked in (rows = 5 i-planes x 16 cin).  Per chunk and per output group:
       - 16 main DRs: (w_hi, w_lo) x broadcast x_hi   (full-precision w)
       - ~9 xlo DRs:  w/16 x x_lo pairs (tile pairs / plane pairs / runt)
     Output-channel groups: A = conv channels (e*10+l) < 128 (128 cols);
     B = the remaining 32 channels pre-contracted with the so3 grid on the
     host (cols = 108 grid-space outputs, accumulated straight into the
     mt=3 so3 psum tile, so no gb matmul / cb copy on device).
     SO(3) grid pooling (relu-weighted second-moment ratio) stays fp16 on
     the PE; the x32 weight scale rides through the a-psum and is folded
     into the relu scale vector.
     Sharding: batch (2) x output-X-slabs (4) -> 8 cores.
     Chunk xr=0 uses a compact 80-row x_lo layout (15 matmuls/group instead
     of 9) so the first chunk does not wait for the big shifted-copy DMAs.
  3. Host: gather per-core slabs into the full (2,16,36,36,36) output.
"""

import os
import sys

for _p in ("/root/.axon_site/_ro/trn_rl_repo", "/opt/trn_rl_repo"):
    if os.path.isdir(_p) and _p not in sys.path:
        sys.path.insert(0, _p)

import numpy as np
import ml_dtypes

import concourse.mybir as mybir
from concourse import bacc
from concourse.tile import TileContext
from concourse.bass_utils import run_bass_kernel_spmd

# Problem constants (hardcoded per harness contract)
ORDER = 2
KS = 5            # conv kernel size
CIN = 16
COUT = 16
EPS = 1e-16
S = 10            # wigner rows
B = 2
D_IN = 40         # input spatial
D_OUT = 36        # output spatial
SLAB = 9          # output X planes per core (36/4)
SLAB_IN = SLAB + KS - 1   # 13 input X planes per core
NCORES = 8
YB = 12           # y-block per chunk
NCHUNK = YB * D_OUT       # 432 columns per matmul chunk
WSCALE = 32.0     # filter pre-scale so fp8 hi/lo split keeps precision

# weight slot layout in wt/wtb tensors: [128, NWSLOT, 2, M]
#   slots 0..14   main (i*3+t): (w_hi, w_lo) for taps t*8..t*8+7 at plane i
#   slots 15..19  xlo tile-pair (i): (w16[i,t0], w16[i,t1])
#   slots 20..21  xlo t2 plane-pair (i=0,2): (w16[i,t2], w16[i+1,t2])
#   slot  22      xlo t2 single (i=4): (w16[4,t2], 0)
NW_MAIN = 0
NW_XPAIR = 15
NW_T2PAIR = 20
NW_T2SINGLE = 22
NWSLOT = 23
NCOLB = 112       # conv-B grid-space cols (108 + 4 zero pad: ldweights
                  # row width must be a multiple of 16)
# runt tensors wr/wrb: [80, 17, 2, M]
#   slot 0 main runt (w_hi, w_lo); slot 1 xlo runt (w16, 0)
#   slots 2..16 the 80-row xlo scheme for chunk xr=0: q = k*3 + i0/2
NR_XLO80 = 2
NRSLOT = 17

F8 = mybir.dt.float8e4
F16 = mybir.dt.float16
F32 = mybir.dt.float32
NPF8 = ml_dtypes.float8_e4m3
DR = mybir.MatmulPerfMode.DoubleRow

_prog_cache = {}


def _conv_chunk(nc, ps, wtx, wrx, xm, xlm, xrh, xrl, xlo80, xr, y0,
                stop_last=True):
    """Emit the conv matmuls of one chunk into psum ps (col count = wtx M)."""
    first = True

    def mm(lhsT, rhs, stop=False):
        nonlocal first
        nc.tensor.matmul(ps[:], lhsT, rhs, start=first, stop=stop,
                         perf_mode=DR)
        first = False

    # main: (w_hi, w_lo) x broadcast x_hi; tile-outer so chunk 0 can start
    # as soon as the first tile's planes have landed
    for t in range(3):
        for i in range(KS):
            rhs = xm[:, t, xr + i:xr + i + 1, y0:y0 + YB, :] \
                .broadcast_to([128, 2, YB, D_OUT])
            mm(wtx[:, NW_MAIN + i * 3 + t, :, :], rhs)
    mm(wrx[:, 0, :, :],
       xrh[:, xr:xr + 1, y0:y0 + YB, :].broadcast_to([80, 2, YB, D_OUT]))

    if xr == 0:
        # compact 80-row xlo: rows (j,cin), k by column offset, i plane-pairs
        q = 0
        for k in range(KS):
            for i0 in (0, 2, 4):
                rhs = xlo80[:, i0:i0 + 2, y0:y0 + YB, k:k + D_OUT]
                mm(wrx[:, NR_XLO80 + q, :, :], rhs,
                   stop=(stop_last and q == 14))
                q += 1
        return

    # xlo: w/16 x x_lo, pairing (t0,t1) tiles per plane
    for i in range(KS):
        mm(wtx[:, NW_XPAIR + i, :, :], xlm[:, 0:2, xr + i, y0:y0 + YB, :])
    # xlo t2 tile: plane pairs (0,1), (2,3); plane 4 solo
    for i in (0, 2):
        s = xr + i
        mm(wtx[:, NW_T2PAIR + i // 2, :, :], xlm[:, 2, s:s + 2, y0:y0 + YB, :])
    s = xr + 4
    mm(wtx[:, NW_T2SINGLE, :, :],
       xlm[:, 2, s:s + 1, y0:y0 + YB, :].broadcast_to([128, 2, YB, D_OUT]))
    # xlo runt tap
    mm(wrx[:, 1, :, :],
       xrl[:, xr:xr + 1, y0:y0 + YB, :].broadcast_to([80, 2, YB, D_OUT]),
       stop=stop_last)


def _build_program():
    """Build the SPMD device program (identical on all 8 cores)."""
    nc = bacc.Bacc("TRN2")

    xm_d = nc.dram_tensor("xm", [128, 3, SLAB_IN, D_OUT, D_OUT], F8,
                          kind="ExternalInput")
    xlm_d = nc.dram_tensor("xlm", [128, 3, SLAB_IN, D_OUT, D_OUT], F8,
                           kind="ExternalInput")
    xrh_d = nc.dram_tensor("xrh", [80, SLAB, D_OUT, D_OUT], F8,
                           kind="ExternalInput")
    xrl_d = nc.dram_tensor("xrl", [80, SLAB, D_OUT, D_OUT], F8,
                           kind="ExternalInput")
    xlo80_d = nc.dram_tensor("xlo80", [80, 6, D_OUT, D_IN], F8,
                             kind="ExternalInput")
    wt_d = nc.dram_tensor("wt", [128, NWSLOT, 2, 128], F8,
                          kind="ExternalInput")
    wtb_d = nc.dram_tensor("wtb", [128, NWSLOT, 2, NCOLB], F8,
                           kind="ExternalInput")
    wr_d = nc.dram_tensor("wr", [80, NRSLOT, 2, 128], F8,
                          kind="ExternalInput")
    wrb_d = nc.dram_tensor("wrb", [80, NRSLOT, 2, NCOLB], F8,
                           kind="ExternalInput")
    ga_d = nc.dram_tensor("ga", [128, 4, 108], F16, kind="ExternalInput")
    wnd_d = nc.dram_tensor("wnd", [108, 64], F16, kind="ExternalInput")
    one8_d = nc.dram_tensor("one8", [108, 4, 2, 16], F8, kind="ExternalInput")
    wvec_d = nc.dram_tensor("wvec", [108, 1], F32, kind="ExternalInput")
    bias_d = nc.dram_tensor("bias", [16, 1], F32, kind="ExternalInput")
    y_d = nc.dram_tensor("y", [16, SLAB, D_OUT, D_OUT], F32,
                         kind="ExternalOutput")

    chunks = [(xr, cy) for xr in range(SLAB) for cy in range(3)]

    with TileContext(nc) as tc:
        with tc.tile_pool(name="const", bufs=1) as cpool, \
             tc.tile_pool(name="work", bufs=4) as wpool, \
             tc.tile_pool(name="casb", bufs=4) as capool, \
             tc.tile_pool(name="rrel", bufs=10) as rpool, \
             tc.tile_pool(name="conv_ps", bufs=2, space="PSUM") as conv_pool, \
             tc.tile_pool(name="a_ps", bufs=3, space="PSUM") as a_pool, \
             tc.tile_pool(name="nd_ps", bufs=2, space="PSUM") as nd_pool, \
             tc.tile_pool(name="den_ps", bufs=1, space="PSUM") as den_pool:

            # ---- resident constants + x tap-copies (all planes SBUF-resident)
            xm = cpool.tile([128, 3, SLAB_IN, D_OUT, D_OUT], F8, tag="xm")
            xlm = cpool.tile([128, 3, SLAB_IN, D_OUT, D_OUT], F8, tag="xlm")
            xrh = cpool.tile([80, SLAB, D_OUT, D_OUT], F8, tag="xrh")
            xrl = cpool.tile([80, SLAB, D_OUT, D_OUT], F8, tag="xrl")
            xlo80 = cpool.tile([80, 6, D_OUT, D_IN], F8, tag="xlo80")
            wt = cpool.tile([128, NWSLOT, 2, 128], F8, tag="wt")
            wtb = cpool.tile([128, NWSLOT, 2, NCOLB], F8, tag="wtb")
            wr = cpool.tile([80, NRSLOT, 2, 128], F8, tag="wr")
            wrb = cpool.tile([80, NRSLOT, 2, NCOLB], F8, tag="wrb")
            gat = cpool.tile([128, 4, 108], F16)
            wndt = cpool.tile([108, 64], F16)
            one8t = cpool.tile([108, 4, 2, 16], F8)
            wvect = cpool.tile([108, 1], F32)
            biast = cpool.tile([16, 1], F32)
            dma_engs = [nc.sync, nc.scalar, nc.gpsimd]

            def _ld_plane(xt, xt_d, t, p0, p1, q):
                dma_engs[q % 3].dma_start(
                    out=xt[:, t, p0:p1].rearrange("p a b c -> p (a b c)"),
                    in_=xt_d[:, t, p0:p1].rearrange("p a b c -> p (a b c)"))

            def _ld_runt(xt, xt_d, p0, p1, q):
                dma_engs[q % 3].dma_start(
                    out=xt[:, p0:p1].rearrange("p a b c -> p (a b c)"),
                    in_=xt_d[:, p0:p1].rearrange("p a b c -> p (a b c)"))

            def _flat(ap):
                return ap.rearrange("p a b c -> p (a b c)")

            # PE warm-up: dummy fp32 matmuls keep the PE busy (and its
            # p-state ramping) while the chunk-0 DMAs land.
            warm = cpool.tile([128, NCHUNK], F32, tag="warm")
            nc.vector.memset(warm[:], 0.0)
            for wi in range(8):
                wps = conv_pool.tile([128, NCHUNK], F32, tag="cps")
                nc.tensor.matmul(wps[:], warm[:, 0:128], warm[:],
                                 start=True, stop=True)

            # chunk-0 critical data first, in big DMAs (per-DMA overhead is
            # ~1.3us).  Chunk-0 critical: weights, xrh plane 0, xm tiles
            # planes 0-4, xlo80.  Then consts, xlm planes 0-4, then the rest
            # in need order.
            # All HWDGE input DMAs ride the sync (SP) queue in need order: SP
            # has no compute, so nothing queues behind the ~630ns/DMA shared
            # HWDGE issue cost.  Small weights + consts go through gpsimd's
            # software DGE (also compute-free early); scalar/ACT stays clear.
            nc.gpsimd.dma_start(out=_flat(wr[:]), in_=_flat(wr_d[:]))
            nc.gpsimd.dma_start(out=_flat(wrb[:]), in_=_flat(wrb_d[:]))
            nc.gpsimd.dma_start(
                out=xlo80.rearrange("p a b c -> p (a b c)"),
                in_=xlo80_d.rearrange("p a b c -> p (a b c)"))
            nc.gpsimd.dma_start(out=gat[:], in_=ga_d[:])
            nc.gpsimd.dma_start(out=wvect[:], in_=wvec_d[:])
            nc.gpsimd.dma_start(out=_flat(xrh[:, 0:1]), in_=_flat(xrh_d[:, 0:1]))
            nc.gpsimd.dma_start(out=wndt[:], in_=wnd_d[:])
            nc.gpsimd.dma_start(out=one8t[:], in_=one8_d[:])
            nc.gpsimd.dma_start(out=biast[:], in_=bias_d[:])
            nc.sync.dma_start(out=_flat(wt[:, 0:NWSLOT]),
                              in_=_flat(wt_d[:, 0:NWSLOT]))
            nc.sync.dma_start(out=_flat(wtb[:, 0:NWSLOT]),
                              in_=_flat(wtb_d[:, 0:NWSLOT]))
            for t in range(3):
                _ld_plane(xm, xm_d, t, 0, KS, 0)
            for t in range(3):
                _ld_plane(xlm, xlm_d, t, 0, KS, 0)
            # tail, in need order (chunk xr needs x plane xr+4, runt plane xr)
            _ld_runt(xrh, xrh_d, 1, 3, 0)
            _ld_runt(xrl, xrl_d, 1, 3, 0)
            for t in range(3):
                _ld_plane(xm, xm_d, t, KS, SLAB, 0)
            for t in range(3):
                _ld_plane(xlm, xlm_d, t, KS, SLAB, 0)
            _ld_runt(xrh, xrh_d, 3, SLAB, 0)
            _ld_runt(xrl, xrl_d, 3, SLAB, 0)
            for t in range(3):
                _ld_plane(xm, xm_d, t, SLAB, SLAB_IN, 0)
            for t in range(3):
                _ld_plane(xlm, xlm_d, t, SLAB, SLAB_IN, 0)

            pending = None
            for (xr, cy) in chunks:
                y0 = cy * YB
                # ---- conv A (128 conv channels), compensated fp8 DR
                cps = conv_pool.tile([128, NCHUNK], F32, tag="cps")
                _conv_chunk(nc, cps, wt, wr, xm, xlm, xrh, xrl, xlo80,
                            xr, y0)
                ca = capool.tile([128, NCHUNK], F16, tag="ca")
                nc.scalar.copy(ca[:], cps[:])

                # ---- conv B: grid-space output accumulated into the mt=3
                # so3 psum tile (so3's gb matmul is folded into the weights)
                aps3f = a_pool.tile([NCOLB, NCHUNK], F32, tag="aps")
                _conv_chunk(nc, aps3f, wtb, wrb, xm, xlm, xrh, xrl, xlo80,
                            xr, y0, stop_last=False)
                aps3 = aps3f[0:108]

                # ---- so3 grid + relu/square (moments lag one chunk)
                rrels, r2s = [], []
                for mt in range(4):
                    if mt < 3:
                        aps = a_pool.tile([108, NCHUNK], F32, tag="aps")
                        nc.tensor.matmul(aps[:], gat[:, mt, :], ca[:],
                                         start=True, stop=True)
                    else:
                        aps = aps3
                        nc.tensor.matmul(aps, gat[:, mt, :], ca[:],
                                         start=False, stop=True)
                    wrel = rpool.tile([108, NCHUNK], F16, tag="rrel")
                    apv = aps[:] if mt < 3 else aps
                    nc.scalar.activation(wrel[:], apv,
                                         mybir.ActivationFunctionType.Relu,
                                         scale=wvect[:, 0:1])
                    w8 = rpool.tile([108, 2, NCHUNK], F8, tag="w8")
                    nc.gpsimd.tensor_copy(w8[:, 0, :], wrel[:])
                    nc.vector.tensor_sub(w8[:, 1, :], wrel[:], w8[:, 0, :])
                    r2 = rpool.tile([108, NCHUNK], F16, tag="r2")
                    nc.vector.tensor_mul(r2[:], wrel[:], wrel[:])
                    rrels.append(w8)
                    r2s.append(r2)
                nd_ps = nd_pool.tile([16, NCHUNK], F32, tag="nd")
                den_ps = den_pool.tile([16, NCHUNK], F32, tag="dn")
                if pending is not None:
                    _emit_moments(nc, wndt, one8t, biast, wpool, y_d, pending)
                pending = (nd_ps, den_ps, rrels, r2s, xr, y0)
            if pending is not None:
                _emit_moments(nc, wndt, one8t, biast, wpool, y_d, pending)

    nc.finalize()
    return nc


def _emit_moments(nc, wndt, one8t, biast, wpool, y_d, st):
    """Emit the 8 moment matmuls + finalize + store for a chunk whose grid
    stage (a/relu/square) was already emitted."""
    nd_ps, den_ps, rrels, r2s, xr, y0 = st
    for mt in range(4):
        wnd_g = wndt[:, mt * 16:(mt + 1) * 16]
        nc.tensor.matmul(nd_ps[:], wnd_g, r2s[mt][:],
                         start=(mt == 0), stop=(mt == 3))
        nc.tensor.matmul(den_ps[:], one8t[:, mt, :, :], rrels[mt][:],
                         start=(mt == 0), stop=(mt == 3), perf_mode=DR)

    num_sb = wpool.tile([16, NCHUNK], F32, tag="num_sb")
    nc.scalar.copy(num_sb[:], nd_ps[:])
    den_sb = wpool.tile([16, NCHUNK], F32, tag="den_sb")
    nc.scalar.activation(den_sb[:], den_ps[:],
         mybir.ActivationFunctionType.Copy,
         bias=EPS)
    recip = wpool.tile([16, NCHUNK], F32, tag="recip")
    nc.vector.reciprocal(recip[:], den_sb[:])
    out_sb = wpool.tile([16, NCHUNK], F32, tag="out_sb")
    nc.vector.tensor_mul(out_sb[:], num_sb[:], recip[:])
    nc.vector.tensor_scalar_add(out_sb[:], out_sb[:], biast[:, 0:1])
    dst = y_d[:, xr].rearrange("p a b -> p (a b)")[
        :, y0 * D_OUT:(y0 + YB) * D_OUT]
    nc.sync.dma_start(out=dst, in_=out_sb[:])


def _synthesize_filter(weight, zeroweight, basis_functions, wig_w, wig_b):
    """Replicate the reference's kernel synthesis in fp32 numpy.

    Returns kern6[l, e, d, i, j, k] of shape (10, 16, 16, 5, 5, 5)."""
    zero_ext = np.concatenate(
        [zeroweight[None, None],
         np.zeros((ORDER ** 2 - 1, 1, CIN, COUT), weight.dtype)], axis=0)
    wfull = np.concatenate([zero_ext, weight], axis=1)       # (4, 10, 16, 16)
    wg = wfull[wig_w]                                        # (10, 10, 16, 16)
    bg = basis_functions[wig_b]                              # (10, 10, 5, 5, 5)
    kern6 = np.einsum("lred,lrijk->ledijk", wg, bg)          # (10,16,16,5,5,5)
    return np.ascontiguousarray(kern6.astype(np.float32))


def _pack_weights(w6, ncols):
    """Pack a [125, CIN, ncols] fp32 filter into wt/wr-style slot tensors."""
    w_hi = w6.astype(NPF8)
    w_lo = (w6 - w_hi.astype(np.float32)).astype(NPF8)
    w_x16 = (w6 / 16).astype(NPF8)

    wt_arr = np.zeros((128, NWSLOT, 2, ncols), NPF8)
    for i in range(KS):
        for t in range(3):
            for jkl in range(8):
                jk = t * 8 + jkl
                j, k = jk // KS, jk % KS
                tap = i * 25 + j * 5 + k
                r0 = jkl * CIN
                wt_arr[r0:r0 + CIN, NW_MAIN + i * 3 + t, 0, :] = w_hi[tap]
                wt_arr[r0:r0 + CIN, NW_MAIN + i * 3 + t, 1, :] = w_lo[tap]
        for sl, t in ((0, 0), (1, 1)):
            for jkl in range(8):
                jk = t * 8 + jkl
                j, k = jk // KS, jk % KS
                tap = i * 25 + j * 5 + k
                r0 = jkl * CIN
                wt_arr[r0:r0 + CIN, NW_XPAIR + i, sl, :] = w_x16[tap]

    def _t2_block(dst_slot, sl, i):
        for jkl in range(8):
            jk = 16 + jkl
            j, k = jk // KS, jk % KS
            tap = i * 25 + j * 5 + k
            r0 = jkl * CIN
            wt_arr[r0:r0 + CIN, dst_slot, sl, :] = w_x16[tap]
    for pi, i in enumerate((0, 2)):
        _t2_block(NW_T2PAIR + pi, 0, i)
        _t2_block(NW_T2PAIR + pi, 1, i + 1)
    _t2_block(NW_T2SINGLE, 0, 4)

    wr_arr = np.zeros((80, NRSLOT, 2, ncols), NPF8)
    for i in range(KS):
        tap = i * 25 + 4 * 5 + 4
        r0 = i * CIN
        wr_arr[r0:r0 + CIN, 0, 0, :] = w_hi[tap]
        wr_arr[r0:r0 + CIN, 0, 1, :] = w_lo[tap]
        wr_arr[r0:r0 + CIN, 1, 0, :] = w_x16[tap]
    # 80-row xlo slots for chunk xr=0: rows (j*16+cin), q = k*3 + i0/2
    q = 0
    for k in range(KS):
        for i0 in (0, 2, 4):
            for j in range(KS):
                r0 = j * CIN
                wr_arr[r0:r0 + CIN, NR_XLO80 + q, 0, :] = \
                    w_x16[i0 * 25 + j * 5 + k]
                if i0 + 1 < KS:
                    wr_arr[r0:r0 + CIN, NR_XLO80 + q, 1, :] = \
                        w_x16[(i0 + 1) * 25 + j * 5 + k]
            q += 1
    return wt_arr, wr_arr


def _host_prep(x, weight, zeroweight, bias, so3basisgrid, w_i,
               basis_functions, wig_w, wig_b):
    kern6 = _synthesize_filter(weight, zeroweight, basis_functions, wig_w, wig_b)

    # w6[tap, cin, col] with tap = i*25 + j*5 + k, col = e*10+l; scaled x32
    w6 = np.ascontiguousarray(
        kern6.transpose(3, 4, 5, 2, 1, 0).reshape(125, CIN, 160)
    ).astype(np.float32) * WSCALE

    g2 = so3basisgrid.reshape(27, S).astype(np.float32)      # raw grid
    g2t = g2.T                                               # [l, mln]

    # B channels (cols 128..159) pre-contracted with the grid: 108 outputs
    # B row r: r=0,1 -> (e12, l8+r); r=2+10*m+l -> (e13+m, l)
    gbmap = np.zeros((32, 108), np.float32)
    for r in range(32):
        if r < 2:
            e, l = 12, 8 + r
        else:
            e, l = 13 + (r - 2) // S, (r - 2) % S
        el2 = e - 12
        gbmap[r, el2 * 27:(el2 + 1) * 27] = g2t[l]
    w6b = np.einsum("tcb,bn->tcn", w6[:, :, 128:], gbmap)
    w6b = np.concatenate(
        [w6b, np.zeros((125, CIN, NCOLB - 108), np.float32)], axis=2)

    wt_arr, wr_arr = _pack_weights(w6[:, :, :128], 128)
    wtb_arr, wrb_arr = _pack_weights(np.ascontiguousarray(w6b), NCOLB)

    # A-tile so3 lhsT: ga[p, mt, el2*27+mln]; p = e*10+l (p < 128)
    ga = np.zeros((128, 4, 108), np.float16)
    for mt in range(4):
        for el2 in range(4):
            e = 4 * mt + el2
            for l in range(S):
                p = e * S + l
                if p < 128:
                    ga[p, mt, el2 * 27:(el2 + 1) * 27] = g2t[l]

    # weighted-moment lhsT: wnd[(el2*27+mln), mt*16+e], e = 4mt+el2
    w_flat = np.asarray(w_i, np.float32)[(np.arange(27) // 3) % 3]
    wnd = np.zeros((108, 4, 16), np.float16)
    one8 = np.zeros((108, 4, 2, 16), NPF8)
    for mt in range(4):
        for el2 in range(4):
            e = 4 * mt + el2
            wnd[el2 * 27:(el2 + 1) * 27, mt, e] = \
                (1.0 / w_flat).astype(np.float16)
            one8[el2 * 27:(el2 + 1) * 27, mt, :, e] = 1.0
    wnd = wnd.reshape(108, 64)
    # the x32 filter scale rides through the a-psum; fold 1/32 into the
    # relu scale so wrel comes out in natural units
    wvec = (np.tile(w_flat, 4) / WSCALE).reshape(108, 1).astype(np.float32)

    bias_arr = np.asarray(bias, np.float32).reshape(16, 1)

    x = np.asarray(x, np.float32)
    xh_all = x.astype(NPF8)
    xl_all = ((x - xh_all.astype(np.float32)) * 16).astype(NPF8)

    in_maps = []
    for c in range(NCORES):
        b, qq = divmod(c, 4)
        p0 = qq * SLAB
        # windowed views: win[cin, p, y, z, j, k] = x[cin, p0+p, y+j, z+k]
        def _wins(arr):
            sl = arr[b, :, p0:p0 + SLAB_IN]        # (16, 13, 40, 40)
            s0, s1, s2, s3 = sl.strides
            return np.lib.stride_tricks.as_strided(
                sl, (CIN, SLAB_IN, D_OUT, D_OUT, KS, KS),
                (s0, s1, s2, s3, s2, s3))
        xm_arr = np.empty((128, 3, SLAB_IN, D_OUT, D_OUT), NPF8)
        xrun = np.empty((2, 80, SLAB, D_OUT, D_OUT), NPF8)
        for hl, arr in enumerate((xh_all, xl_all)):
            w = _wins(arr)
            # main tiles: row (jkl*16+cin) of tile t = tap jk = t*8+jkl
            wv = w.transpose(4, 5, 0, 1, 2, 3).reshape(
                25, CIN, SLAB_IN, D_OUT, D_OUT)
            tiles = wv[:24].reshape(3, 8, CIN, SLAB_IN, D_OUT, D_OUT) \
                .reshape(3, 128, SLAB_IN, D_OUT, D_OUT) \
                .transpose(1, 0, 2, 3, 4)
            if hl == 0:
                xm_arr[:] = tiles
            else:
                xlm_arr = np.ascontiguousarray(tiles)
            # runt: row (i*16+cin) at out-plane xr = x[cin, xr+i, y+4, z+4]
            rw = w[:, :, :, :, 4, 4]               # (16, 13, 36, 36)
            for i in range(KS):
                xrun[hl, i * CIN:(i + 1) * CIN] = rw[:, i:i + SLAB]
        # 80-row xlo for chunk 0: rows (j*16+cin), planes 0..5, z full
        xlo80 = np.empty((80, 6, D_OUT, D_IN), NPF8)
        sl = xl_all[b, :, p0:p0 + 6]               # (16, 6, 40, 40)
        for j in range(KS):
            xlo80[j * CIN:(j + 1) * CIN] = sl[:, :, j:j + D_OUT, :]
        in_maps.append({
            "xm": np.ascontiguousarray(xm_arr),
            "xlm": xlm_arr,
            "xrh": np.ascontiguousarray(xrun[0]),
            "xrl": np.ascontiguousarray(xrun[1]),
            "xlo80": np.ascontiguousarray(xlo80),
            "wt": wt_arr,
            "wtb": wtb_arr,
            "wr": wr_arr,
            "wrb": wrb_arr,
            "ga": np.ascontiguousarray(ga),
            "wnd": np.ascontiguousarray(wnd),
            "one8": np.ascontiguousarray(one8),
            "wvec": np.ascontiguousarray(wvec),
            "bias": bias_arr,
        })
    return in_maps


def _run(inputs, trace=False, **run_kwargs):
    inputs = {k: np.asarray(v) for k, v in inputs.items()}
    in_maps = _host_prep(**inputs)
    if "nc" not in _prog_cache:
        _prog_cache["nc"] = _build_program()
    nc = _prog_cache["nc"]
    try:
        res = run_bass_kernel_spmd(nc, in_maps, core_ids=list(range(NCORES)),
                                   trace=trace, **run_kwargs)
    except ModuleNotFoundError as e:
        if "axon_hooks" not in str(e):
            raise
        # Tracing requested (e.g. BASS_TRACE=1) but this axon client has no
        # NTFF profile hook - rerun with tracing disabled.
        os.environ["BASS_NEVER_TRACE"] = "1"
        res = run_bass_kernel_spmd(nc, in_maps, core_ids=list(range(NCORES)),
                                   trace=False, **run_kwargs)
    out = np.empty((B, COUT, D_OUT, D_OUT, D_OUT), np.float32)
    for c in range(NCORES):
        b, qq = divmod(c, 4)
        out[b, :, qq * SLAB:(qq + 1) * SLAB] = res.results[c]["y"]
    return out, res


def kernel(**inputs):
    out, _ = _run(inputs)
    return out


# revision 37
# speedup vs baseline: 1.1695x; 1.1315x over previous
"""Trainium2 Bass kernel for nn_InvLocalPatOrientConvolution.

Computation:
  1. Host: synthesize the 160-channel 5x5x5 conv filter (scaled x32), split
     weights and input into fp8-e4m3 hi/lo pairs (w = w_hi + w_lo exactly at
     fp8 resolution; x = x_hi + x_lo/16), lay out per-core operands.
  2. Device (8 NeuronCores, SPMD): VALID 3D conv as fp8 DoubleRow PE matmuls.
     The conv contraction (125 taps x 16 cin = 2000 rows) is packed onto the
     full 128 SBUF partitions: the host materializes (j,k)-shifted copies of
     x so that rows = 8 (j,k)-taps x 16 cin per tile (3 tiles = taps 0..23),
     plus an 80-row "runt" tensor holding tap 24 = (j=4,k=4) with the i-shift
     baked in (rows = 5 i-planes x 16 cin).  Per chunk and per output group:
       - 16 main DRs: (w_hi, w_lo) x broadcast x_hi   (full-precision w)
       - ~9 xlo DRs:  w/16 x x_lo pairs (tile pairs / plane pairs / runt)
     Output-channel groups: A = conv channels (e*10+l) < 128 (128 cols);
     B = the remaining 32 channels pre-contracted with the so3 grid on the
     host (cols = 108 grid-space outputs, accumulated straight into the
     mt=3 so3 psum tile, so no gb matmul / cb copy on device).
     SO(3) grid pooling (relu-weighted second-moment ratio) stays fp16 on
     the PE; the x32 weight scale rides through the a-psum and is folded
     into the relu scale vector.
     Sharding: batch (2) x output-X-slabs (4) -> 8 cores.
     Chunk xr=0 uses a compact 80-row x_lo layout (15 matmuls/group instead
     of 9) so the first chunk does not wait for the big shifted-copy DMAs.
  3. Host: gather per-core slabs into the full (2,16,36,36,36) output.
"""

import os
import sys

for _p in ("/root/.axon_site/_ro/trn_rl_repo", "/opt/trn_rl_repo"):
    if os.path.isdir(_p) and _p not in sys.path:
        sys.path.insert(0, _p)

import numpy as np
import ml_dtypes

import concourse.mybir as mybir
from concourse import bacc
from concourse.tile import TileContext
from concourse.bass_utils import run_bass_kernel_spmd

# Problem constants (hardcoded per harness contract)
ORDER = 2
KS = 5            # conv kernel size
CIN = 16
COUT = 16
EPS = 1e-16
S = 10            # wigner rows
B = 2
D_IN = 40         # input spatial
D_OUT = 36        # output spatial
SLAB = 9          # output X planes per core (36/4)
SLAB_IN = SLAB + KS - 1   # 13 input X planes per core
NCORES = 8
YB = 12           # y-block per chunk
NCHUNK = YB * D_OUT       # 432 columns per matmul chunk
WSCALE = 32.0     # filter pre-scale so fp8 hi/lo split keeps precision

# weight slot layout in wt/wtb tensors: [128, NWSLOT, 2, M]
#   slots 0..14   main (i*3+t): (w_hi, w_lo) for taps t*8..t*8+7 at plane i
#   slots 15..19  xlo tile-pair (i): (w16[i,t0], w16[i,t1])
#   slots 20..21  xlo t2 plane-pair (i=0,2): (w16[i,t2], w16[i+1,t2])
#   slot  22      xlo t2 single (i=4): (w16[4,t2], 0)
NW_MAIN = 0
NW_XPAIR = 15
NW_T2PAIR = 20
NW_T2SINGLE = 22
NWSLOT = 23
NCOLB = 112       # conv-B grid-space cols (108 + 4 zero pad: ldweights
                  # row width must be a multiple of 16)
# runt tensors wr/wrb: [80, 17, 2, M]
#   slot 0 main runt (w_hi, w_lo); slot 1 xlo runt (w16, 0)
#   slots 2..16 the 80-row xlo scheme for chunk xr=0: q = k*3 + i0/2
NR_XLO80 = 2
NRSLOT = 17

F8 = mybir.dt.float8e4
F16 = mybir.dt.float16
F32 = mybir.dt.float32
NPF8 = ml_dtypes.float8_e4m3
DR = mybir.MatmulPerfMode.DoubleRow

_prog_cache = {}


def _conv_chunk(nc, ps, wtx, wrx, xm, xlm, xrh, xrl, xlo80, xr, y0,
                stop_last=True, use80=False, yb=YB):
    """Emit the conv matmuls of one chunk into psum ps (col count = wtx M)."""
    first = True

    def mm(lhsT, rhs, stop=False):
        nonlocal first
        nc.tensor.matmul(ps[:], lhsT, rhs, start=first, stop=stop,
                         perf_mode=DR)
        first = False

    # main: (w_hi, w_lo) x broadcast x_hi; tile-outer so chunk 0 can start
    # as soon as the first tile's planes have landed
    for t in range(3):
        for i in range(KS):
            rhs = xm[:, t, xr + i:xr + i + 1, y0:y0 + yb, :] \
                .broadcast_to([128, 2, yb, D_OUT])
            mm(wtx[:, NW_MAIN + i * 3 + t, :, :], rhs)
    mm(wrx[:, 0, :, :],
       xrh[:, xr:xr + 1, y0:y0 + yb, :].broadcast_to([80, 2, yb, D_OUT]))

    if use80:
        # compact 80-row xlo: rows (j,cin), k by column offset, i plane-pairs
        q = 0
        for k in range(KS):
            for i0 in (0, 2, 4):
                rhs = xlo80[:, i0:i0 + 2, y0:y0 + yb, k:k + D_OUT]
                mm(wrx[:, NR_XLO80 + q, :, :], rhs,
                   stop=(stop_last and q == 14))
                q += 1
        return

    # xlo: w/16 x x_lo, pairing (t0,t1) tiles per plane
    for i in range(KS):
        mm(wtx[:, NW_XPAIR + i, :, :], xlm[:, 0:2, xr + i, y0:y0 + yb, :])
    # xlo t2 tile: plane pairs (0,1), (2,3); plane 4 solo
    for i in (0, 2):
        s = xr + i
        mm(wtx[:, NW_T2PAIR + i // 2, :, :], xlm[:, 2, s:s + 2, y0:y0 + yb, :])
    s = xr + 4
    mm(wtx[:, NW_T2SINGLE, :, :],
       xlm[:, 2, s:s + 1, y0:y0 + yb, :].broadcast_to([128, 2, yb, D_OUT]))
    # xlo runt tap
    mm(wrx[:, 1, :, :],
       xrl[:, xr:xr + 1, y0:y0 + yb, :].broadcast_to([80, 2, yb, D_OUT]),
       stop=stop_last)


def _build_program(has_bias=True):
    """Build the SPMD device program (identical on all 8 cores)."""
    nc = bacc.Bacc("TRN2")

    xm_d = nc.dram_tensor("xm", [128, 3, SLAB_IN, D_OUT, D_OUT], F8,
                          kind="ExternalInput")
    xlm_d = nc.dram_tensor("xlm", [128, 3, SLAB_IN, D_OUT, D_OUT], F8,
                           kind="ExternalInput")
    xrh_d = nc.dram_tensor("xrh", [80, SLAB, D_OUT, D_OUT], F8,
                           kind="ExternalInput")
    xrl_d = nc.dram_tensor("xrl", [80, SLAB, D_OUT, D_OUT], F8,
                           kind="ExternalInput")
    xlo80_d = nc.dram_tensor("xlo80", [80, 6, D_OUT, D_IN], F8,
                             kind="ExternalInput")
    wt_d = nc.dram_tensor("wt", [128, NWSLOT, 2, 128], F8,
                          kind="ExternalInput")
    wtb_d = nc.dram_tensor("wtb", [128, NWSLOT, 2, NCOLB], F8,
                           kind="ExternalInput")
    wr_d = nc.dram_tensor("wr", [80, NRSLOT, 2, 128], F8,
                          kind="ExternalInput")
    wrb_d = nc.dram_tensor("wrb", [80, NRSLOT, 2, NCOLB], F8,
                           kind="ExternalInput")
    ga_d = nc.dram_tensor("ga", [128, 4, 108], F16, kind="ExternalInput")
    wnd_d = nc.dram_tensor("wnd", [108, 64], F16, kind="ExternalInput")
    one8_d = nc.dram_tensor("one8", [108, 4, 2, 16], F8, kind="ExternalInput")
    wvec_d = nc.dram_tensor("wvec", [108, 1], F32, kind="ExternalInput")
    bias_d = nc.dram_tensor("bias", [16, 1], F32, kind="ExternalInput")
    y_d = nc.dram_tensor("y", [16, SLAB, D_OUT, D_OUT], F32,
                         kind="ExternalOutput")

    # last chunk split into 3 mini-chunks so the exposed final moments
    # chain runs on small tiles
    chunks = [(xr, cy * YB, YB) for xr in range(SLAB) for cy in range(3)]
    chunks = chunks[:-1] + [(SLAB - 1, 2 * YB + 6 * m, 6) for m in range(2)]

    with TileContext(nc) as tc:
        with tc.tile_pool(name="const", bufs=1) as cpool, \
             tc.tile_pool(name="work", bufs=4) as wpool, \
             tc.tile_pool(name="casb", bufs=4) as capool, \
             tc.tile_pool(name="rrel", bufs=10) as rpool, \
             tc.tile_pool(name="conv_ps", bufs=2, space="PSUM") as conv_pool, \
             tc.tile_pool(name="a_ps", bufs=3, space="PSUM") as a_pool, \
             tc.tile_pool(name="nd_ps", bufs=2, space="PSUM") as nd_pool, \
             tc.tile_pool(name="den_ps", bufs=1, space="PSUM") as den_pool:

            # ---- resident constants + x tap-copies (all planes SBUF-resident)
            xm = cpool.tile([128, 3, SLAB_IN, D_OUT, D_OUT], F8, tag="xm")
            xlm = cpool.tile([128, 3, SLAB_IN, D_OUT, D_OUT], F8, tag="xlm")
            xrh = cpool.tile([80, SLAB, D_OUT, D_OUT], F8, tag="xrh")
            xrl = cpool.tile([80, SLAB, D_OUT, D_OUT], F8, tag="xrl")
            xlo80 = cpool.tile([80, 6, D_OUT, D_IN], F8, tag="xlo80")
            wt = cpool.tile([128, NWSLOT, 2, 128], F8, tag="wt")
            wtb = cpool.tile([128, NWSLOT, 2, NCOLB], F8, tag="wtb")
            wr = cpool.tile([80, NRSLOT, 2, 128], F8, tag="wr")
            wrb = cpool.tile([80, NRSLOT, 2, NCOLB], F8, tag="wrb")
            gat = cpool.tile([128, 4, 108], F16)
            wndt = cpool.tile([108, 64], F16)
            one8t = cpool.tile([108, 4, 2, 16], F8)
            wvect = cpool.tile([108, 1], F32)
            biast = cpool.tile([16, 1], F32)
            dma_engs = [nc.sync, nc.scalar, nc.gpsimd]

            def _ld_plane(xt, xt_d, t, p0, p1, q):
                dma_engs[q % 3].dma_start(
                    out=xt[:, t, p0:p1].rearrange("p a b c -> p (a b c)"),
                    in_=xt_d[:, t, p0:p1].rearrange("p a b c -> p (a b c)"))

            def _ld_runt(xt, xt_d, p0, p1, q):
                dma_engs[q % 3].dma_start(
                    out=xt[:, p0:p1].rearrange("p a b c -> p (a b c)"),
                    in_=xt_d[:, p0:p1].rearrange("p a b c -> p (a b c)"))

            def _flat(ap):
                return ap.rearrange("p a b c -> p (a b c)")

            # PE warm-up: dummy fp32 matmuls keep the PE busy (and its
            # p-state ramping) while the chunk-0 DMAs land.
            warm = cpool.tile([128, 1], F32, tag="warm")
            nc.vector.memset(warm[:], 0.0)
            for wi in range(6):
                wps = conv_pool.tile([128, NCHUNK], F32, tag="cps")
                nc.tensor.matmul(wps[0:1, :], warm[:, 0:1],
                                 warm[:, 0:1].broadcast_to([128, NCHUNK]),
                                 start=True, stop=True)

            # chunk-0 critical data first: main weights, then narrow y-band
            # (rows 0-15) slices of the xm tiles so chunk (0,0) can start
            # early, then xlo80/runts; full planes + everything else follow.
            # All HWDGE input DMAs ride the sync (SP) queue in need order: SP
            # has no compute, so nothing queues behind the ~630ns/DMA shared
            # HWDGE issue cost.  Small weights go through gpsimd's software
            # DGE (also compute-free early); scalar/ACT stays clear.
            nc.gpsimd.dma_start(out=_flat(wr[:, 0:2]), in_=_flat(wr_d[:, 0:2]))
            nc.gpsimd.dma_start(out=_flat(wrb[:, 0:2]),
                                in_=_flat(wrb_d[:, 0:2]))
            nc.gpsimd.dma_start(out=_flat(wr[:, 2:NRSLOT]),
                                in_=_flat(wr_d[:, 2:NRSLOT]))
            nc.gpsimd.dma_start(out=_flat(wrb[:, 2:NRSLOT]),
                                in_=_flat(wrb_d[:, 2:NRSLOT]))
            nc.gpsimd.dma_start(out=_flat(xrh[:, 0:1]), in_=_flat(xrh_d[:, 0:1]))
            nc.gpsimd.dma_start(out=_flat(xrl[:, 0:1]), in_=_flat(xrl_d[:, 0:1]))
            nc.sync.dma_start(out=_flat(wt[:, 0:NW_XPAIR]),
                              in_=_flat(wt_d[:, 0:NW_XPAIR]))
            YB0 = YB + KS - 1   # y rows needed by chunk (0,*,cy=0)
            for t in range(3):
                dma_engs[0].dma_start(out=xm[:, t, 0:KS, 0:YB0],
                                      in_=xm_d[:, t, 0:KS, 0:YB0])
            nc.sync.dma_start(out=_flat(wtb[:, 0:NW_XPAIR]),
                              in_=_flat(wtb_d[:, 0:NW_XPAIR]))
            nc.sync.dma_start(out=xlo80[:, :, 0:YB0],
                              in_=xlo80_d[:, :, 0:YB0])
            nc.sync.dma_start(out=gat[:], in_=ga_d[:])
            nc.sync.dma_start(out=wvect[:], in_=wvec_d[:])
            # full planes 0-4 (rows 16-35 for cy=1,2 of xr=0 + re-send band)
            for t in range(3):
                dma_engs[0].dma_start(out=xm[:, t, 0:KS, YB0:],
                                      in_=xm_d[:, t, 0:KS, YB0:])
            nc.sync.dma_start(out=xlo80[:, :, YB0:],
                              in_=xlo80_d[:, :, YB0:])
            nc.sync.dma_start(out=_flat(wt[:, NW_XPAIR:NWSLOT]),
                              in_=_flat(wt_d[:, NW_XPAIR:NWSLOT]))
            nc.sync.dma_start(out=_flat(wtb[:, NW_XPAIR:NWSLOT]),
                              in_=_flat(wtb_d[:, NW_XPAIR:NWSLOT]))
            nc.sync.dma_start(out=wndt[:], in_=wnd_d[:])
            nc.sync.dma_start(out=one8t[:], in_=one8_d[:])
            nc.sync.dma_start(out=biast[:], in_=bias_d[:])
            for t in range(3):
                _ld_plane(xlm, xlm_d, t, 0, KS, 0)
            # tail, in need order (chunk xr needs x plane xr+4, runt plane xr)
            for t in range(3):
                _ld_plane(xm, xm_d, t, KS, KS + 2, 0)
            for t in range(3):
                _ld_plane(xlm, xlm_d, t, KS, KS + 2, 0)
            _ld_runt(xrh, xrh_d, 1, 3, 0)
            _ld_runt(xrl, xrl_d, 1, 3, 0)
            for t in range(3):
                _ld_plane(xm, xm_d, t, KS + 2, SLAB, 0)
            for t in range(3):
                _ld_plane(xlm, xlm_d, t, KS + 2, SLAB, 0)
            _ld_runt(xrh, xrh_d, 3, SLAB, 0)
            _ld_runt(xrl, xrl_d, 3, SLAB, 0)
            for t in range(3):
                _ld_plane(xm, xm_d, t, SLAB, SLAB_IN, 0)
            for t in range(3):
                _ld_plane(xlm, xlm_d, t, SLAB, SLAB_IN, 0)

            pending = None
            for ci, (xr, y0, yb) in enumerate(chunks):
                nck = yb * D_OUT
                use80 = (ci <= 2)
                # ---- conv A (128 conv channels), compensated fp8 DR
                cps = conv_pool.tile([128, nck], F32, tag="cps")
                _conv_chunk(nc, cps, wt, wr, xm, xlm, xrh, xrl, xlo80,
                            xr, y0, use80=use80, yb=yb)
                ca = capool.tile([128, nck], F16, tag="ca")
                nc.scalar.copy(ca[:], cps[:])

                # ---- conv B: grid-space output accumulated into the mt=3
                # so3 psum tile (so3's gb matmul is folded into the weights)
                aps3f = a_pool.tile([NCOLB, nck], F32, tag="aps")
                _conv_chunk(nc, aps3f, wtb, wrb, xm, xlm, xrh, xrl, xlo80,
                            xr, y0, use80=use80, yb=yb)
                aps3 = aps3f[0:108]

                # ---- so3 grid + relu/square (moments lag one chunk)
                rrels, r2s = [], []
                for mt in range(4):
                    if mt < 3:
                        aps = a_pool.tile([108, nck], F32, tag="aps")
                        nc.tensor.matmul(aps[:], gat[:, mt, :], ca[:],
                                         start=True, stop=True)
                    else:
                        aps = aps3
                    wrel = rpool.tile([108, nck], F16, tag="rrel")
                    apv = aps[:] if mt < 3 else aps
                    nc.scalar.activation(wrel[:], apv,
                                         mybir.ActivationFunctionType.Relu,
                                         scale=wvect[:, 0:1])
                    w8 = rpool.tile([108, 2, nck], F8, tag="w8")
                    nc.gpsimd.tensor_copy(w8[:, 0, :], wrel[:])
                    nc.vector.tensor_sub(w8[:, 1, :], wrel[:], w8[:, 0, :])
                    r2 = rpool.tile([108, nck], F16, tag="r2")
                    nc.vector.tensor_mul(r2[:], wrel[:], wrel[:])
                    rrels.append(w8)
                    r2s.append(r2)
                nd_ps = nd_pool.tile([16, nck], F32, tag="nd")
                den_ps = den_pool.tile([16, nck], F32, tag="dn")
                if pending is not None:
                    se = nc.scalar if ci >= len(chunks) - 1 else None
                    _emit_moments(nc, wndt, one8t, biast, wpool, y_d, pending,
                                  store_eng=se, has_bias=has_bias)
                pending = (nd_ps, den_ps, rrels, r2s, xr, y0, yb)
            if pending is not None:
                _emit_moments(nc, wndt, one8t, biast, wpool, y_d, pending,
                              store_eng=nc.sync, has_bias=has_bias)

    nc.finalize()
    return nc


def _emit_moments(nc, wndt, one8t, biast, wpool, y_d, st, store_eng=None,
                  has_bias=True):
    """Emit the 8 moment matmuls + finalize + store for a chunk whose grid
    stage (a/relu/square) was already emitted."""
    nd_ps, den_ps, rrels, r2s, xr, y0, yb = st
    nck = yb * D_OUT
    for mt in range(4):
        wnd_g = wndt[:, mt * 16:(mt + 1) * 16]
        nc.tensor.matmul(nd_ps[:], wnd_g, r2s[mt][:],
                         start=(mt == 0), stop=(mt == 3))
        nc.tensor.matmul(den_ps[:], one8t[:, mt, :, :], rrels[mt][:],
                         start=(mt == 0), stop=(mt == 3), perf_mode=DR)

    num_sb = wpool.tile([16, nck], F32, tag="num_sb")
    nc.scalar.copy(num_sb[:], nd_ps[:])
    den_sb = wpool.tile([16, nck], F32, tag="den_sb")
    nc.scalar.activation(den_sb[:], den_ps[:],
         mybir.ActivationFunctionType.Copy,
         bias=EPS)
    recip = wpool.tile([16, nck], F32, tag="recip")
    nc.vector.reciprocal_approx_fast(recip[:], den_sb[:])
    out_sb = wpool.tile([16, nck], F32, tag="out_sb")
    nc.vector.tensor_mul(out_sb[:], num_sb[:], recip[:])
    if has_bias:
        nc.vector.tensor_scalar_add(out_sb[:], out_sb[:], biast[:, 0:1])
    dst = y_d[:, xr].rearrange("p a b -> p (a b)")[
        :, y0 * D_OUT:(y0 + yb) * D_OUT]
    (store_eng or nc.sync).dma_start(out=dst, in_=out_sb[:])


def _synthesize_filter(weight, zeroweight, basis_functions, wig_w, wig_b):
    """Replicate the reference's kernel synthesis in fp32 numpy.

    Returns kern6[l, e, d, i, j, k] of shape (10, 16, 16, 5, 5, 5)."""
    zero_ext = np.concatenate(
        [zeroweight[None, None],
         np.zeros((ORDER ** 2 - 1, 1, CIN, COUT), weight.dtype)], axis=0)
    wfull = np.concatenate([zero_ext, weight], axis=1)       # (4, 10, 16, 16)
    wg = wfull[wig_w]                                        # (10, 10, 16, 16)
    bg = basis_functions[wig_b]                              # (10, 10, 5, 5, 5)
    kern6 = np.einsum("lred,lrijk->ledijk", wg, bg)          # (10,16,16,5,5,5)
    return np.ascontiguousarray(kern6.astype(np.float32))


def _pack_weights(w6, ncols):
    """Pack a [125, CIN, ncols] fp32 filter into wt/wr-style slot tensors."""
    w_hi = w6.astype(NPF8)
    w_lo = (w6 - w_hi.astype(np.float32)).astype(NPF8)
    w_x16 = (w6 / 16).astype(NPF8)

    wt_arr = np.zeros((128, NWSLOT, 2, ncols), NPF8)
    for i in range(KS):
        for t in range(3):
            for jkl in range(8):
                jk = t * 8 + jkl
                j, k = jk // KS, jk % KS
                tap = i * 25 + j * 5 + k
                r0 = jkl * CIN
                wt_arr[r0:r0 + CIN, NW_MAIN + i * 3 + t, 0, :] = w_hi[tap]
                wt_arr[r0:r0 + CIN, NW_MAIN + i * 3 + t, 1, :] = w_lo[tap]
        for sl, t in ((0, 0), (1, 1)):
            for jkl in range(8):
                jk = t * 8 + jkl
                j, k = jk // KS, jk % KS
                tap = i * 25 + j * 5 + k
                r0 = jkl * CIN
                wt_arr[r0:r0 + CIN, NW_XPAIR + i, sl, :] = w_x16[tap]

    def _t2_block(dst_slot, sl, i):
        for jkl in range(8):
            jk = 16 + jkl
            j, k = jk // KS, jk % KS
            tap = i * 25 + j * 5 + k
            r0 = jkl * CIN
            wt_arr[r0:r0 + CIN, dst_slot, sl, :] = w_x16[tap]
    for pi, i in enumerate((0, 2)):
        _t2_block(NW_T2PAIR + pi, 0, i)
        _t2_block(NW_T2PAIR + pi, 1, i + 1)
    _t2_block(NW_T2SINGLE, 0, 4)

    wr_arr = np.zeros((80, NRSLOT, 2, ncols), NPF8)
    for i in range(KS):
        tap = i * 25 + 4 * 5 + 4
        r0 = i * CIN
        wr_arr[r0:r0 + CIN, 0, 0, :] = w_hi[tap]
        wr_arr[r0:r0 + CIN, 0, 1, :] = w_lo[tap]
        wr_arr[r0:r0 + CIN, 1, 0, :] = w_x16[tap]
    # 80-row xlo slots for chunk xr=0: rows (j*16+cin), q = k*3 + i0/2
    q = 0
    for k in range(KS):
        for i0 in (0, 2, 4):
            for j in range(KS):
                r0 = j * CIN
                wr_arr[r0:r0 + CIN, NR_XLO80 + q, 0, :] = \
                    w_x16[i0 * 25 + j * 5 + k]
                if i0 + 1 < KS:
                    wr_arr[r0:r0 + CIN, NR_XLO80 + q, 1, :] = \
                        w_x16[(i0 + 1) * 25 + j * 5 + k]
            q += 1
    return wt_arr, wr_arr


def _host_prep(x, weight, zeroweight, bias, so3basisgrid, w_i,
               basis_functions, wig_w, wig_b):
    kern6 = _synthesize_filter(weight, zeroweight, basis_functions, wig_w, wig_b)

    # w6[tap, cin, col] with tap = i*25 + j*5 + k, col = e*10+l; scaled x32
    w6 = np.ascontiguousarray(
        kern6.transpose(3, 4, 5, 2, 1, 0).reshape(125, CIN, 160)
    ).astype(np.float32) * WSCALE

    g2 = so3basisgrid.reshape(27, S).astype(np.float32)      # raw grid
    g2t = g2.T                                               # [l, mln]

    # B channels (cols 128..159) pre-contracted with the grid: 108 outputs
    # B row r: r=0,1 -> (e12, l8+r); r=2+10*m+l -> (e13+m, l)
    gbmap = np.zeros((32, 108), np.float32)
    for r in range(32):
        if r < 2:
            e, l = 12, 8 + r
        else:
            e, l = 13 + (r - 2) // S, (r - 2) % S
        el2 = e - 12
        gbmap[r, el2 * 27:(el2 + 1) * 27] = g2t[l]
    w6b = np.einsum("tcb,bn->tcn", w6[:, :, 128:], gbmap)
    # fold in the (e12, l0-7) channels (conv-A cols 120-127), which feed
    # only so3 group mt=3 -- this removes the mt3 ga matmul entirely
    ga3 = np.zeros((8, 108), np.float32)
    for l in range(8):
        ga3[l, 0:27] = g2t[l]
    w6b += np.einsum("tcb,bn->tcn", w6[:, :, 120:128], ga3)
    w6b = np.concatenate(
        [w6b, np.zeros((125, CIN, NCOLB - 108), np.float32)], axis=2)

    wt_arr, wr_arr = _pack_weights(w6[:, :, :128], 128)
    wtb_arr, wrb_arr = _pack_weights(np.ascontiguousarray(w6b), NCOLB)

    # A-tile so3 lhsT: ga[p, mt, el2*27+mln]; p = e*10+l (p < 128)
    ga = np.zeros((128, 4, 108), np.float16)
    for mt in range(4):
        for el2 in range(4):
            e = 4 * mt + el2
            for l in range(S):
                p = e * S + l
                if p < 128:
                    ga[p, mt, el2 * 27:(el2 + 1) * 27] = g2t[l]

    # weighted-moment lhsT: wnd[(el2*27+mln), mt*16+e], e = 4mt+el2
    w_flat = np.asarray(w_i, np.float32)[(np.arange(27) // 3) % 3]
    wnd = np.zeros((108, 4, 16), np.float16)
    one8 = np.zeros((108, 4, 2, 16), NPF8)
    for mt in range(4):
        for el2 in range(4):
            e = 4 * mt + el2
            wnd[el2 * 27:(el2 + 1) * 27, mt, e] = \
                (1.0 / w_flat).astype(np.float16)
            one8[el2 * 27:(el2 + 1) * 27, mt, :, e] = 1.0
    wnd = wnd.reshape(108, 64)
    # the x32 filter scale rides through the a-psum; fold 1/32 into the
    # relu scale so wrel comes out in natural units
    wvec = (np.tile(w_flat, 4) / WSCALE).reshape(108, 1).astype(np.float32)

    bias_arr = np.asarray(bias, np.float32).reshape(16, 1)

    x = np.asarray(x, np.float32)
    xh_all = x.astype(NPF8)
    xl_all = ((x - xh_all.astype(np.float32)) * 16).astype(NPF8)

    in_maps = []
    for c in range(NCORES):
        b, qq = divmod(c, 4)
        p0 = qq * SLAB
        # windowed views: win[cin, p, y, z, j, k] = x[cin, p0+p, y+j, z+k]
        def _wins(arr):
            sl = arr[b, :, p0:p0 + SLAB_IN]        # (16, 13, 40, 40)
            s0, s1, s2, s3 = sl.strides
            return np.lib.stride_tricks.as_strided(
                sl, (CIN, SLAB_IN, D_OUT, D_OUT, KS, KS),
                (s0, s1, s2, s3, s2, s3))
        xm_arr = np.empty((128, 3, SLAB_IN, D_OUT, D_OUT), NPF8)
        xrun = np.empty((2, 80, SLAB, D_OUT, D_OUT), NPF8)
        for hl, arr in enumerate((xh_all, xl_all)):
            w = _wins(arr)
            # main tiles: row (jkl*16+cin) of tile t = tap jk = t*8+jkl
            wv = w.transpose(4, 5, 0, 1, 2, 3).reshape(
                25, CIN, SLAB_IN, D_OUT, D_OUT)
            tiles = wv[:24].reshape(3, 8, CIN, SLAB_IN, D_OUT, D_OUT) \
                .reshape(3, 128, SLAB_IN, D_OUT, D_OUT) \
                .transpose(1, 0, 2, 3, 4)
            if hl == 0:
                xm_arr[:] = tiles
            else:
                xlm_arr = np.ascontiguousarray(tiles)
            # runt: row (i*16+cin) at out-plane xr = x[cin, xr+i, y+4, z+4]
            rw = w[:, :, :, :, 4, 4]               # (16, 13, 36, 36)
            for i in range(KS):
                xrun[hl, i * CIN:(i + 1) * CIN] = rw[:, i:i + SLAB]
        # 80-row xlo for chunk 0: rows (j*16+cin), planes 0..5, z full
        xlo80 = np.empty((80, 6, D_OUT, D_IN), NPF8)
        sl = xl_all[b, :, p0:p0 + 6]               # (16, 6, 40, 40)
        for j in range(KS):
            xlo80[j * CIN:(j + 1) * CIN] = sl[:, :, j:j + D_OUT, :]
        in_maps.append({
            "xm": np.ascontiguousarray(xm_arr),
            "xlm": xlm_arr,
            "xrh": np.ascontiguousarray(xrun[0]),
            "xrl": np.ascontiguousarray(xrun[1]),
            "xlo80": np.ascontiguousarray(xlo80),
            "wt": wt_arr,
            "wtb": wtb_arr,
            "wr": wr_arr,
            "wrb": wrb_arr,
            "ga": np.ascontiguousarray(ga),
            "wnd": np.ascontiguousarray(wnd),
            "one8": np.ascontiguousarray(one8),
            "wvec": np.ascontiguousarray(wvec),
            "bias": bias_arr,
        })
    return in_maps


def _run(inputs, trace=False, **run_kwargs):
    inputs = {k: np.asarray(v) for k, v in inputs.items()}
    in_maps = _host_prep(**inputs)
    has_bias = bool(np.any(np.asarray(inputs["bias"]) != 0))
    key = f"nc{int(has_bias)}"
    if key not in _prog_cache:
        _prog_cache[key] = _build_program(has_bias)
    nc = _prog_cache[key]
    try:
        res = run_bass_kernel_spmd(nc, in_maps, core_ids=list(range(NCORES)),
                                   trace=trace, **run_kwargs)
    except ModuleNotFoundError as e:
        if "axon_hooks" not in str(e):
            raise
        # Tracing requested (e.g. BASS_TRACE=1) but this axon client has no
        # NTFF profile hook - rerun with tracing disabled.
        os.environ["BASS_NEVER_TRACE"] = "1"
        res = run_bass_kernel_spmd(nc, in_maps, core_ids=list(range(NCORES)),
                                   trace=False, **run_kwargs)
    out = np.empty((B, COUT, D_OUT, D_OUT, D_OUT), np.float32)
    for c in range(NCORES):
        b, qq = divmod(c, 4)
        out[b, :, qq * SLAB:(qq + 1) * SLAB] = res.results[c]["y"]
    return out, res


def kernel(**inputs):
    out, _ = _run(inputs)
    return out
